# revision 29
# baseline (speedup 1.0000x reference)
"""CrossAttention Trainium2 kernel, v3 (PE-tiling + HAM-warm restructure).

Full inputs in, full output out. Sharding: data-parallel over batch (B=2),
tensor-parallel over heads (16 heads -> 4 groups of 4), 8 cores total.
Each core computes attention for its (batch, 4-head group) and a partial
output projection in bf16; the host sums the 4 partials per batch and adds
bout.

v3 changes vs v2 (trace-driven):
  - Head-PAIR score matmuls: the two heads of a kT/qTs tile live on PE
    row-groups 0 and 64 (K=64 each); issuing them back-to-back runs them
    concurrently on the PE (row tiling), halving score time.
  - QE=512 q-blocks; the pair's scores land in one [128, 1024] PSUM tile
    (2 banks) -> ONE exp instruction per (pair, ct) keeps the ACT
    instruction count at 128 while PSUM fits (st 4 + av 2 + sml 2 banks).
  - Normalization off the PE critical path: av [65, 512] PSUM tiles are
    copied to SBUF immediately (frees the bank in ~0.6us), then
    reciprocal/broadcast/multiply run concurrently with the next pair's
    stream.  v2 held av through a 5.8us single-lane reciprocal, stalling
    the PE ~6us at every head boundary and re-throttling HAM to half
    clock (148us of K=4/8 measured).
  - hp-outer loop (all q-blocks for head-pair 0, then pair 1): kT[1]
    projection + out-projections ride later slack; v-projection is split
    per head-pair and JIT'd inside the e0 passes.
  - Column-slab cT DMAs so k-projection (and the stream) starts after the
    first 0.5MB slab instead of the full 4MB.
"""

import numpy as np
import ml_dtypes
from collections import deque

import concourse.bass as bass
import concourse.tile as tile
from concourse import bacc, mybir
from concourse import bass2jax

BF16 = mybir.dt.bfloat16
F32 = mybir.dt.float32
NPBF16 = ml_dtypes.bfloat16

B, MODEL, H, D = 2, 1024, 16, 64
LQ, LC = 2048, 2048
N_CORES = 8
GROUPS = 4            # head groups = cores per batch
HPC = H // GROUPS     # heads per core (4)
HD = HPC * D          # 256 head-dims per core
SCALE = D ** -0.5
EXPF = mybir.ActivationFunctionType.Exp


def build_nc(lq=LQ, lc=LC, reps=1):
    NT = MODEL // 128         # m-contraction chunks (8)
    DT = HD // 128            # d-tiles / head-pairs (2)
    CT = lc // 128            # context chunks (16)
    QE = 512                  # q-block (free dim of scores)
    NQE = lq // QE            # 4
    CS = lc // 512            # 512-col context slabs (4)

    nc = bacc.Bacc(trn_type="TRN2", target_bir_lowering=False, debug=False,
                   num_devices=N_CORES)

    cT_d = nc.dram_tensor("cT", [MODEL, lc], BF16, kind="ExternalInput").ap()
    qT_d = nc.dram_tensor("qT", [MODEL, lq], BF16, kind="ExternalInput").ap()
    # packed: per chunk a: cols [a*512 : a*512+256] = wk_a, [+256 : +512] = wq_a
    wkq_d = nc.dram_tensor("wkq", [128, NT * 512], BF16,
                           kind="ExternalInput").ap()
    wv_d = nc.dram_tensor("wv", [128, NT * HD], BF16, kind="ExternalInput").ap()
    wvb_d = nc.dram_tensor("wvb", [1, HD], BF16, kind="ExternalInput").ap()
    wo_d = nc.dram_tensor("wo", [128, DT * MODEL], BF16,
                          kind="ExternalInput").ap()
    # cols: [bq_t0, bq_t1, bk_t0, bk_t1]
    bqk_d = nc.dram_tensor("bqk", [128, 2 * DT], F32, kind="ExternalInput").ap()
    out_d = nc.dram_tensor("outp", [lq, MODEL], BF16, kind="ExternalOutput").ap()

    with tile.TileContext(nc) as tc:
        with tc.tile_pool(name="w", bufs=1) as wp, \
             tc.tile_pool(name="acts", bufs=1) as acp, \
             tc.tile_pool(name="ptp", bufs=7) as ptp, \
             tc.tile_pool(name="avs", bufs=6) as avs, \
             tc.tile_pool(name="nrm", bufs=4) as nrm, \
             tc.tile_pool(name="atp", bufs=4) as atp, \
             tc.tile_pool(name="osb", bufs=3) as osb, \
             tc.tile_pool(name="stp", bufs=2, space="PSUM") as stp, \
             tc.tile_pool(name="avp", bufs=1, space="PSUM") as avp, \
             tc.tile_pool(name="sml", bufs=2, space="PSUM") as sml:

            # ---- activation-table warmup (hide LoadActFuncSet in the ramp)
            warm_in = acp.tile([1, 8], F32, name="warm_in", tag="warm_in")
            warm_out = acp.tile([1, 8], BF16, name="warm_out", tag="warm_out")
            nc.vector.memset(warm_in[:], 0.0)
            nc.scalar.activation(warm_out[:], warm_in[:], EXPF, scale=SCALE)

            # ---- SBUF tiles
            wkq_sb = wp.tile([128, NT * 512], BF16, name="wkq", tag="wkq")
            wv_sb = wp.tile([128, NT * HD], BF16, name="wv", tag="wv")
            wvb_sb = wp.tile([1, HD], BF16, name="wvb", tag="wvb")
            wo_sb = wp.tile([128, DT * MODEL], BF16, name="wo", tag="wo")
            bqk_sb = wp.tile([128, 2 * DT], F32, name="bqk", tag="bqk")
            cTi = [acp.tile([128, lc], BF16, name=f"cTi{a}", tag=f"cTi{a}")
                   for a in range(NT)]
            qTi = [acp.tile([128, lq], BF16, name=f"qTi{a}", tag=f"qTi{a}")
                   for a in range(NT)]
            ones_c = acp.tile([1, lc], BF16, name="ones_c", tag="ones_c")
            nc.vector.memset(ones_c[:], 1.0)

            kT = [acp.tile([128, lc], BF16, name=f"kT{t}", tag=f"kT{t}")
                  for t in range(DT)]
            qTs = [[acp.tile([128, QE], BF16, name=f"qTs{t}_{e}",
                             tag=f"qTs{t}_{e}") for e in range(NQE)]
                   for t in range(DT)]
            v_sb = [acp.tile([128, HPC * (D + 1)], BF16, name=f"v{ct}",
                             tag=f"v{ct}") for ct in range(CT)]
            attn = [[acp.tile([128, QE], BF16, name=f"attn{t}_{e}",
                              tag=f"attn{t}_{e}") for e in range(NQE)]
                    for t in range(DT)]

            # ---- input DMAs, priority order (arrival order = issue order).
            # wkq in chunks so kproj slab 0 starts after the first 128KB;
            # cT slab 0 ahead of qT/wv: kproj slab 0 gates the stream start.
            for a in range(NT):
                nc.sync.dma_start(wkq_sb[:, a * 512:(a + 1) * 512],
                                  wkq_d[:, a * 512:(a + 1) * 512])
            nc.sync.dma_start(bqk_sb[:], bqk_d[:, :])
            for a in range(NT):
                nc.sync.dma_start(cTi[a][:, 0:512],
                                  cT_d[a * 128:(a + 1) * 128, 0:512])
            for a in range(NT):
                nc.sync.dma_start(qTi[a][:, 0:QE], qT_d[a * 128:(a + 1) * 128,
                                                        0:QE])
            nc.sync.dma_start(wv_sb[:], wv_d[:, :])
            nc.sync.dma_start(wvb_sb[:], wvb_d[:, :])
            # remaining cT in 512-col slabs so kproj/vproj chase arrivals.
            for s in range(1, CS):
                for a in range(NT):
                    nc.sync.dma_start(cTi[a][:, s * 512:(s + 1) * 512],
                                      cT_d[a * 128:(a + 1) * 128,
                                           s * 512:(s + 1) * 512])
            nc.sync.dma_start(wo_sb[:], wo_d[:, :])
            for a in range(NT):
                nc.sync.dma_start(qTi[a][:, QE:lq],
                                  qT_d[a * 128:(a + 1) * 128, QE:lq])

            def wk_ap(a, t):
                return wkq_sb[:, a * 512 + t * 128:a * 512 + (t + 1) * 128]

            def wq_ap(a, t):
                return wkq_sb[:, a * 512 + 256 + t * 128:
                              a * 512 + 256 + (t + 1) * 128]

            def emit_all(reps_i):
                # producer gens (qproj/kproj) — must complete before their
                # consumer block's scores are EMITTED (in-order PE queue).
                prod = deque()
                # consumer gens (outproj) — their LDWEIGHTS reads attn, so
                # drain them only well after the normalize that writes it.
                late = deque()

                # Credit-based, GEN-ATOMIC drains: once a generator starts,
                # run it to completion (credit may go negative and is repaid
                # by later pumps).  A generator left mid-flight holds an sml
                # PSUM slot; any other sml allocation would then reuse that
                # slot and wait on the held gen's later-emitted reads — a
                # head-of-line deadlock in the in-order PE queue (v3.2 bug:
                # NRT_EXEC_UNIT_UNRECOVERABLE).
                credit = {"prod": 0, "late": 0}

                def drain(q, key, n):
                    if not q:
                        credit[key] = 0
                        return
                    credit[key] += n
                    while credit[key] > 0 and q:
                        g = q[0]
                        try:
                            while True:
                                next(g)
                                credit[key] -= 1
                        except StopIteration:
                            q.popleft()

                def qproj(t, e):
                    """q-projection for (d-tile t, q-block e), inline."""
                    ps = sml.tile([128, QE], F32, name="sml_ps", tag="sml")
                    for a in range(NT):
                        nc.tensor.matmul(
                            ps[:], wq_ap(a, t),
                            qTi[a][:, e * QE:(e + 1) * QE],
                            start=(a == 0), stop=(a == NT - 1))
                    nc.vector.tensor_scalar_add(
                        qTs[t][e][:], ps[:], bqk_sb[:, t:t + 1])

                def gen_qproj(t, e):
                    ps = sml.tile([128, QE], F32, name="sml_ps", tag="sml")
                    for a in range(NT):
                        nc.tensor.matmul(
                            ps[:], wq_ap(a, t),
                            qTi[a][:, e * QE:(e + 1) * QE],
                            start=(a == 0), stop=(a == NT - 1))
                        yield
                    nc.vector.tensor_scalar_add(
                        qTs[t][e][:], ps[:], bqk_sb[:, t:t + 1])

                def kproj_slab(t, qb):
                    """k-projection d-tile t for 512-col slab qb, inline."""
                    ps = sml.tile([128, 512], F32, name="sml_ps", tag="sml")
                    for a in range(NT):
                        nc.tensor.matmul(
                            ps[:], wk_ap(a, t),
                            cTi[a][:, qb * 512:(qb + 1) * 512],
                            start=(a == 0), stop=(a == NT - 1))
                    nc.vector.tensor_scalar_add(
                        kT[t][:, qb * 512:(qb + 1) * 512], ps[:],
                        bqk_sb[:, DT + t:DT + t + 1])

                def gen_kproj_slab(t, qb):
                    ps = sml.tile([128, 512], F32, name="sml_ps", tag="sml")
                    for a in range(NT):
                        nc.tensor.matmul(
                            ps[:], wk_ap(a, t),
                            cTi[a][:, qb * 512:(qb + 1) * 512],
                            start=(a == 0), stop=(a == NT - 1))
                        yield
                    nc.vector.tensor_scalar_add(
                        kT[t][:, qb * 512:(qb + 1) * 512], ps[:],
                        bqk_sb[:, DT + t:DT + t + 1])

                def vproj_half(ct, hp):
                    """v-projection for context chunk ct, head pair hp."""
                    HHD = 128  # two heads * 64 d
                    ps = sml.tile([128, 512], F32, name="sml_ps", tag="sml")
                    for a in range(NT):
                        nc.tensor.matmul(
                            ps[:, 0:HHD], cTi[a][:, ct * 128:(ct + 1) * 128],
                            wv_sb[:, a * HD + hp * HHD:
                                  a * HD + (hp + 1) * HHD],
                            start=(a == 0), stop=False)
                    nc.tensor.matmul(
                        ps[:, 0:HHD], ones_c[:, ct * 128:(ct + 1) * 128],
                        wvb_sb[:, hp * HHD:(hp + 1) * HHD],
                        start=False, stop=True)
                    vg = v_sb[ct].rearrange("p (g x) -> p g x", x=D + 1)
                    nc.vector.tensor_copy(
                        vg[:, 2 * hp:2 * hp + 2, 0:D],
                        ps[:, 0:HHD].rearrange("p (g x) -> p g x", x=D))
                    nc.vector.memset(vg[:, 2 * hp:2 * hp + 2, D:D + 1], 1.0)

                def gen_outproj(e, qt):
                    """out-projection for query tile qt of q-block e."""
                    ostg = osb.tile([128, MODEL], BF16, name="ostg", tag="ostg")
                    for nh in range(MODEL // 512):
                        ps = sml.tile([128, 512], F32, name="sml_ps",
                                      tag="sml")
                        for t in range(DT):
                            nc.tensor.matmul(
                                ps[:], attn[t][e][:, qt * 128:(qt + 1) * 128],
                                wo_sb[:, t * MODEL + nh * 512:
                                      t * MODEL + (nh + 1) * 512],
                                start=(t == 0), stop=(t == DT - 1))
                            yield
                        nc.vector.tensor_copy(
                            ostg[:, nh * 512:(nh + 1) * 512], ps[:])
                    nc.sync.dma_start(
                        out_d[(e * QE + qt * 128):(e * QE + (qt + 1) * 128), :],
                        ostg[:])

                # ---------- ramp: k-proj t0 slab 0 first (gates stream),
                # then q-proj (t0, e0), then remaining k-proj t0 slabs.
                kproj_slab(0, 0)
                qproj(0, 0)
                for qb in range(1, CS):
                    kproj_slab(0, qb)

                # ---------- producer queue (drained in PE slack) ----------
                # hp0 phase consumers: qTs[0][e] (e>=1), kT[1], qTs[1][*]
                for e in range(1, NQE):
                    prod.append(gen_qproj(0, e))
                for qb in range(CS):
                    prod.append(gen_kproj_slab(1, qb))
                prod.append(gen_qproj(1, 0))
                for e in range(1, NQE):
                    prod.append(gen_qproj(1, e))

                def norm_step(hp, e, asb, h, c):
                    """One normalize micro-step: 128-col chunk c of head h.
                    Chunked so (a) the scheduler's ~12x-optimistic reciprocal
                    cost model can only misplace ~1us of dependent work, and
                    (b) each outproj qt tile depends ONLY on its own chunk."""
                    f0, f1 = c * 128, (c + 1) * 128
                    def step():
                        rz = nrm.tile([1, 128], BF16, name="rz", tag="rz")
                        with nc.allow_low_precision(
                                reason="1/Z in bf16 (~0.4%), inside the "
                                       "2e-2 budget"):
                            nc.vector.reciprocal(rz[:], asb[64:65, f0:f1])
                        rb = nrm.tile([64, 128], BF16, name="rb", tag="rb")
                        nc.gpsimd.partition_broadcast(rb[:], rz[:])
                        # muls on GPSIMD: keeps the DVE chain recips-only
                        # (the serial tail latency) and off the DVE queue.
                        if h == 0:
                            nc.gpsimd.tensor_mul(
                                attn[hp][e][0:64, f0:f1], asb[0:64, f0:f1],
                                rb[:])
                        else:
                            at = atp.tile([64, 128], BF16, name="at",
                                          tag="at")
                            nc.gpsimd.tensor_mul(at[:], asb[0:64, f0:f1],
                                                 rb[:])
                            nc.sync.dma_start(attn[hp][e][64:128, f0:f1],
                                              at[:])
                    return step

                norm_q = deque()   # pending normalize micro-steps (FIFO)

                # ---------- attention stream: hp outer, q-blocks inner ----
                for hp in range(DT):
                    for e in range(NQE):
                        av = [avp.tile([65, QE], F32, name=f"av{h}",
                                       tag=f"av{h}") for h in range(2)]
                        pend = deque()   # (pt, ct) awaiting AV emission

                        def emit_av(hp=hp, av=av, pend=pend):
                            ppt, pct = pend.popleft()
                            for h in range(2):
                                nc.tensor.matmul(
                                    av[h][:],
                                    v_sb[pct][:, (2 * hp + h) * (D + 1):
                                              (2 * hp + h + 1) * (D + 1)],
                                    ppt[:, h * QE:(h + 1) * QE],
                                    start=(pct == 0), stop=(pct == CT - 1))

                        for ct in range(CT):
                            # deferred normalize micro-steps, 2/ct once the
                            # boundary-critical DVE work is behind us (not
                            # in e==0 blocks: vproj owns the DVE there)
                            if e > 0 and ct >= 2:
                                for _ in range(2):
                                    if norm_q:
                                        norm_q.popleft()()
                            st = stp.tile([128, 2 * QE], F32, name="st",
                                          tag="st")
                            nc.tensor.matmul(
                                st[:, 0:QE],
                                kT[hp][0:64, ct * 128:(ct + 1) * 128],
                                qTs[hp][e][0:64, :],
                                start=True, stop=True)
                            nc.tensor.matmul(
                                st[:, QE:2 * QE],
                                kT[hp][64:128, ct * 128:(ct + 1) * 128],
                                qTs[hp][e][64:128, :],
                                start=True, stop=True)
                            pt = ptp.tile([128, 2 * QE], BF16, name="pt",
                                          tag="pt")
                            nc.scalar.activation(pt[:], st[:], EXPF,
                                                 scale=SCALE)
                            # JIT v-projection inside the first q-block of
                            # each hp phase (AV(ct) trails by 4 steps).
                            if e == 0:
                                vproj_half(ct, hp)
                            pend.append((pt, ct))
                            if len(pend) > 4:
                                emit_av()
                            if e > 0:
                                if ct < 8:
                                    drain(prod, "prod", 2)
                                elif ct >= 14:
                                    drain(late, "late", 5)
                        while pend:
                            emit_av()
                        # ---- normalize pair: copies NOW (free the PSUM av
                        # banks in ~1.2us); recip/bcast/mul deferred as
                        # chunked micro-steps, interleaved h0/h1 per chunk
                        # so outproj qt tiles unblock in qt order.
                        asbs = []
                        for h in range(2):
                            asb = avs.tile([65, QE], BF16, name="asb",
                                           tag="asb")
                            nc.vector.tensor_copy(asb[:], av[h][:])
                            asbs.append(asb)
                        for c in range(QE // 128):
                            for h in range(2):
                                norm_q.append(norm_step(hp, e, asbs[h], h, c))
                        # Boundary filler: outproj gens queued a block ago
                        # are fully ready (their norm finished mid-block) —
                        # they keep the PE busy while the last exps drain.
                        drain(late, "late", 10)
                        # Block-end drain: the next block's qTs/kT producers
                        # MUST complete before its first scores are emitted
                        # (the PE queue is in-order; a later-queued producer
                        # would deadlock an earlier-queued consumer).
                        drain(prod, "prod", 10)
                        # after pair 1 of block e: queue its out-projection
                        if hp == DT - 1:
                            for qt in range(QE // 128):
                                late.append(gen_outproj(e, qt))

                # ---------- tail ----------
                while norm_q:
                    norm_q.popleft()()
                drain(late, "late", 10 ** 9)
                drain(prod, "prod", 10 ** 9)

            for r in range(reps):
                emit_all(r)

    nc.compile()
    return nc


def _prep_inputs(query, context, Wq, bq, Wkv, bkv, Wout, lq, lc):
    """Host-side shard/cast/pack. Returns in_maps for 8 cores."""
    NT = MODEL // 128
    DT = HD // 128
    Wkv_r = np.asarray(Wkv, np.float32).reshape(MODEL, H, D, 2)
    bkv_r = np.asarray(bkv, np.float32).reshape(H, D, 2)
    Wq = np.asarray(Wq, np.float32)
    bq = np.asarray(bq, np.float32)
    Wout = np.asarray(Wout, np.float32)

    qT = [np.ascontiguousarray(np.asarray(query[b], np.float32).T).astype(NPBF16)
          for b in range(B)]
    cT = [np.ascontiguousarray(np.asarray(context[b], np.float32).T).astype(NPBF16)
          for b in range(B)]

    grp = []
    for g in range(GROUPS):
        hs = slice(g * HPC, (g + 1) * HPC)
        wq_c = Wq[:, g * HD:(g + 1) * HD]                      # [M, HD]
        wk_c = Wkv_r[:, hs, :, 0].reshape(MODEL, HD)           # [M, HD]
        wv_c = Wkv_r[:, hs, :, 1].reshape(MODEL, HD)           # [M, HD]
        bv_c = bkv_r[hs, :, 1].reshape(1, HD)
        wo_c = Wout[g * HD:(g + 1) * HD, :]                    # [HD, M]
        bq_c = bq[g * HD:(g + 1) * HD]
        bk_c = bkv_r[hs, :, 0].reshape(HD)

        # wkq packed [128, NT*512]: chunk a -> [wk_a | wq_a]
        wkq = np.empty((128, NT * 512), np.float32)
        for a in range(NT):
            wkq[:, a * 512:a * 512 + 256] = wk_c[a * 128:(a + 1) * 128, :]
            wkq[:, a * 512 + 256:(a + 1) * 512] = wq_c[a * 128:(a + 1) * 128, :]
        # wv packed [128, NT*HD]: chunk a at cols a*HD
        wvp = np.empty((128, NT * HD), np.float32)
        for a in range(NT):
            wvp[:, a * HD:(a + 1) * HD] = wv_c[a * 128:(a + 1) * 128, :]
        # wo packed [128, DT*MODEL]: d-tile t at cols t*MODEL
        wop = np.empty((128, DT * MODEL), np.float32)
        for t in range(DT):
            wop[:, t * MODEL:(t + 1) * MODEL] = wo_c[t * 128:(t + 1) * 128, :]
        # biases [128, 2*DT]: cols [bq_t0, bq_t1, bk_t0, bk_t1]
        bqk = np.empty((128, 2 * DT), np.float32)
        for t in range(DT):
            bqk[:, t] = bq_c[t * 128:(t + 1) * 128]
            bqk[:, DT + t] = bk_c[t * 128:(t + 1) * 128]
        grp.append((wkq.astype(NPBF16), wvp.astype(NPBF16),
                    np.ascontiguousarray(bv_c, np.float32).astype(NPBF16),
                    wop.astype(NPBF16), bqk))

    in_maps = []
    for c in range(N_CORES):
        b, g = c // GROUPS, c % GROUPS
        wkq, wvp, wvb, wop, bqk = grp[g]
        in_maps.append({
            "qT": qT[b], "cT": cT[b],
            "wkq": wkq, "wv": wvp, "wvb": wvb, "wo": wop, "bqk": bqk,
        })
    return in_maps


def _reduce_out(results, bout, lq):
    bout = np.asarray(bout, np.float32)
    out = np.empty((B, lq, MODEL), np.float32)
    for b in range(B):
        acc = results[b * GROUPS]["outp"].astype(np.float32)
        for g in range(1, GROUPS):
            acc += results[b * GROUPS + g]["outp"].astype(np.float32)
        out[b] = acc + bout
    return out


class Runner:
    """Cached-jit PJRT executor for the SPMD bass kernel (axon path).

    Mirrors bass2jax.run_bass_via_pjrt's multi-core branch, but builds the
    jitted sharded callable once so repeated calls reuse the compiled
    executable (for steady-state timing) and inputs can be pre-placed on
    device.
    """

    def __init__(self, nc, n_cores=N_CORES):
        import jax
        from jax.sharding import Mesh, PartitionSpec, NamedSharding
        from jax.experimental.shard_map import shard_map

        bass2jax.install_neuronx_cc_hook()
        assert nc.dbg_addr is None
        part_name = (nc.partition_id_tensor.name
                     if nc.partition_id_tensor else None)

        in_names, out_names, out_avals, zero_outs = [], [], [], []
        for alloc in nc.m.functions[0].allocations:
            if not isinstance(alloc, mybir.MemoryLocationSet):
                continue
            name = alloc.memorylocations[0].name
            if alloc.kind == "ExternalInput":
                if name != part_name:
                    in_names.append(name)
            elif alloc.kind == "ExternalOutput":
                shape = tuple(alloc.tensor_shape)
                dtype = mybir.dt.np(alloc.dtype)
                out_names.append(name)
                out_avals.append(jax.core.ShapedArray(shape, dtype))
                zero_outs.append(np.zeros(shape, dtype))
        self.n_params = len(in_names)
        self.in_names = list(in_names)
        self.out_names = out_names
        self.out_avals = out_avals
        self.zero_outs = zero_outs
        all_names = tuple(
            in_names + out_names + ([part_name] if part_name else []))

        def _body(*args):
            operands = list(args)
            if part_name is not None:
                operands.append(bass2jax.partition_id_tensor())
            outs = bass2jax._bass_exec_p.bind(
                *operands,
                out_avals=tuple(out_avals),
                in_names=all_names,
                out_names=tuple(out_names),
                lowering_input_output_aliases=(),
                sim_require_finite=True,
                sim_require_nnan=True,
                nc=nc,
            )
            return tuple(outs)

        devices = jax.devices()[:n_cores]
        assert len(devices) == n_cores
        self.n_cores = n_cores
        self.mesh = Mesh(np.asarray(devices), ("core",))
        self.sharding = NamedSharding(self.mesh, PartitionSpec("core"))
        n_args = self.n_params + len(out_names)
        self.fn = jax.jit(
            shard_map(_body, mesh=self.mesh,
                      in_specs=(PartitionSpec("core"),) * n_args,
                      out_specs=(PartitionSpec("core"),) * len(out_names),
                      check_rep=False),
            keep_unused=True,
        )
        self._jax = jax

    def put(self, in_maps):
        """Concatenate per-core inputs on axis 0 and place on devices."""
        jax = self._jax
        args = []
        for name in self.in_names:
            arr = np.concatenate(
                [np.asarray(m[name]) for m in in_maps], axis=0)
            args.append(jax.device_put(arr, self.sharding))
        for z in self.zero_outs:
            zz = np.zeros((self.n_cores * z.shape[0], *z.shape[1:]), z.dtype)
            args.append(jax.device_put(zz, self.sharding))
        return args

    def call(self, dev_args):
        outs = self.fn(*dev_args)
        self._jax.block_until_ready(outs)
        return outs

    def gather(self, outs):
        """outs -> list (per core) of {name: np.ndarray}."""
        res = []
        for c in range(self.n_cores):
            d = {}
            for i, name in enumerate(self.out_names):
                full = np.asarray(outs[i])
                d[name] = full.reshape(
                    self.n_cores, *self.out_avals[i].shape)[c]
            res.append(d)
        return res


_CACHE = {}


def _get_runner(lq, lc):
    key = (lq, lc)
    if key not in _CACHE:
        _CACHE[key] = Runner(build_nc(lq, lc))
    return _CACHE[key]


def run(query, context, Wq, bq, Wkv, bkv, Wout, bout):
    lq, lc = query.shape[1], context.shape[1]
    runner = _get_runner(lq, lc)
    in_maps = _prep_inputs(query, context, Wq, bq, Wkv, bkv, Wout, lq, lc)
    dev_args = runner.put(in_maps)
    outs = runner.call(dev_args)
    results = runner.gather(outs)
    return _reduce_out(results, bout, lq), runner, dev_args


def kernel(query, context, Wq, bq, Wkv, bkv, Wout, bout):
    out, _, _ = run(query, context, Wq, bq, Wkv, bkv, Wout, bout)
    return out


# revision 30
# speedup vs baseline: 2.2129x; 2.2129x over previous
"""CrossAttention Trainium2 kernel, v3 (PE-tiling + HAM-warm restructure).

Full inputs in, full output out. Sharding: data-parallel over batch (B=2),
tensor-parallel over heads (16 heads -> 4 groups of 4), 8 cores total.
Each core computes attention for its (batch, 4-head group) and a partial
output projection in bf16; the host sums the 4 partials per batch and adds
bout.

v3 changes vs v2 (trace-driven):
  - Head-PAIR score matmuls: the two heads of a kT/qTs tile live on PE
    row-groups 0 and 64 (K=64 each); issuing them back-to-back runs them
    concurrently on the PE (row tiling), halving score time.
  - QE=512 q-blocks; the pair's scores land in one [128, 1024] PSUM tile
    (2 banks) -> ONE exp instruction per (pair, ct) keeps the ACT
    instruction count at 128 while PSUM fits (st 4 + av 2 + sml 2 banks).
  - Normalization off the PE critical path: av [65, 512] PSUM tiles are
    copied to SBUF immediately (frees the bank in ~0.6us), then
    reciprocal/broadcast/multiply run concurrently with the next pair's
    stream.  v2 held av through a 5.8us single-lane reciprocal, stalling
    the PE ~6us at every head boundary and re-throttling HAM to half
    clock (148us of K=4/8 measured).
  - hp-outer loop (all q-blocks for head-pair 0, then pair 1): kT[1]
    projection + out-projections ride later slack; v-projection is split
    per head-pair and JIT'd inside the e0 passes.
  - Column-slab cT DMAs so k-projection (and the stream) starts after the
    first 0.5MB slab instead of the full 4MB.
"""

import numpy as np
import ml_dtypes
from collections import deque

import concourse.bass as bass
import concourse.tile as tile
from concourse import bacc, mybir
from concourse import bass2jax

BF16 = mybir.dt.bfloat16
F32 = mybir.dt.float32
NPBF16 = ml_dtypes.bfloat16

B, MODEL, H, D = 2, 1024, 16, 64
LQ, LC = 2048, 2048
N_CORES = 8
GROUPS = 4            # head groups = cores per batch
HPC = H // GROUPS     # heads per core (4)
HD = HPC * D          # 256 head-dims per core
SCALE = D ** -0.5
EXPF = mybir.ActivationFunctionType.Exp


def build_nc(lq=LQ, lc=LC, reps=1):
    NT = MODEL // 128         # m-contraction chunks (8)
    DT = HD // 128            # d-tiles / head-pairs (2)
    CT = lc // 128            # context chunks (16)
    QE = 512                  # q-block (free dim of scores)
    NQE = lq // QE            # 4
    CS = lc // 512            # 512-col context slabs (4)

    nc = bacc.Bacc(trn_type="TRN2", target_bir_lowering=False, debug=False,
                   num_devices=N_CORES)

    cT_d = nc.dram_tensor("cT", [MODEL, lc], BF16, kind="ExternalInput").ap()
    qT_d = nc.dram_tensor("qT", [MODEL, lq], BF16, kind="ExternalInput").ap()
    # packed: per chunk a: cols [a*512 : a*512+256] = wk_a, [+256 : +512] = wq_a
    wkq_d = nc.dram_tensor("wkq", [128, NT * 512], BF16,
                           kind="ExternalInput").ap()
    wv_d = nc.dram_tensor("wv", [128, NT * HD], BF16, kind="ExternalInput").ap()
    wvb_d = nc.dram_tensor("wvb", [1, HD], BF16, kind="ExternalInput").ap()
    wo_d = nc.dram_tensor("wo", [128, DT * MODEL], BF16,
                          kind="ExternalInput").ap()
    # cols: [bq_t0, bq_t1, bk_t0, bk_t1]
    bqk_d = nc.dram_tensor("bqk", [128, 2 * DT], F32, kind="ExternalInput").ap()
    out_d = nc.dram_tensor("outp", [lq, MODEL], BF16, kind="ExternalOutput").ap()

    with tile.TileContext(nc) as tc:
        with tc.tile_pool(name="w", bufs=1) as wp, \
             tc.tile_pool(name="acts", bufs=1) as acp, \
             tc.tile_pool(name="ptp", bufs=7) as ptp, \
             tc.tile_pool(name="avs", bufs=6) as avs, \
             tc.tile_pool(name="nrm", bufs=4) as nrm, \
             tc.tile_pool(name="atp", bufs=4) as atp, \
             tc.tile_pool(name="osb", bufs=3) as osb, \
             tc.tile_pool(name="stp", bufs=2, space="PSUM") as stp, \
             tc.tile_pool(name="avp", bufs=1, space="PSUM") as avp, \
             tc.tile_pool(name="sml", bufs=2, space="PSUM") as sml:

            # ---- activation-table warmup (hide LoadActFuncSet in the ramp)
            warm_in = acp.tile([1, 8], F32, name="warm_in", tag="warm_in")
            warm_out = acp.tile([1, 8], BF16, name="warm_out", tag="warm_out")
            nc.vector.memset(warm_in[:], 0.0)
            nc.scalar.activation(warm_out[:], warm_in[:], EXPF, scale=SCALE)

            # ---- SBUF tiles
            wkq_sb = wp.tile([128, NT * 512], BF16, name="wkq", tag="wkq")
            wv_sb = wp.tile([128, NT * HD], BF16, name="wv", tag="wv")
            wvb_sb = wp.tile([1, HD], BF16, name="wvb", tag="wvb")
            wo_sb = wp.tile([128, DT * MODEL], BF16, name="wo", tag="wo")
            bqk_sb = wp.tile([128, 2 * DT], F32, name="bqk", tag="bqk")
            cTi = [acp.tile([128, lc], BF16, name=f"cTi{a}", tag=f"cTi{a}")
                   for a in range(NT)]
            qTi = [acp.tile([128, lq], BF16, name=f"qTi{a}", tag=f"qTi{a}")
                   for a in range(NT)]
            ones_c = acp.tile([1, lc], BF16, name="ones_c", tag="ones_c")
            nc.vector.memset(ones_c[:], 1.0)

            kT = [acp.tile([128, lc], BF16, name=f"kT{t}", tag=f"kT{t}")
                  for t in range(DT)]
            qTs = [[acp.tile([128, QE], BF16, name=f"qTs{t}_{e}",
                             tag=f"qTs{t}_{e}") for e in range(NQE)]
                   for t in range(DT)]
            v_sb = [acp.tile([128, HPC * (D + 1)], BF16, name=f"v{ct}",
                             tag=f"v{ct}") for ct in range(CT)]
            attn = [[acp.tile([128, QE], BF16, name=f"attn{t}_{e}",
                              tag=f"attn{t}_{e}") for e in range(NQE)]
                    for t in range(DT)]

            # ---- input DMAs, priority order (arrival order = issue order).
            # wkq in chunks so kproj slab 0 starts after the first 128KB;
            # cT slab 0 ahead of qT/wv: kproj slab 0 gates the stream start.
            for a in range(NT):
                nc.sync.dma_start(wkq_sb[:, a * 512:(a + 1) * 512],
                                  wkq_d[:, a * 512:(a + 1) * 512])
            nc.sync.dma_start(bqk_sb[:], bqk_d[:, :])
            for a in range(NT):
                nc.sync.dma_start(cTi[a][:, 0:512],
                                  cT_d[a * 128:(a + 1) * 128, 0:512])
            for a in range(NT):
                nc.sync.dma_start(qTi[a][:, 0:QE], qT_d[a * 128:(a + 1) * 128,
                                                        0:QE])
            nc.sync.dma_start(wv_sb[:], wv_d[:, :])
            nc.sync.dma_start(wvb_sb[:], wvb_d[:, :])
            # remaining cT in 512-col slabs so kproj/vproj chase arrivals.
            for s in range(1, CS):
                for a in range(NT):
                    nc.sync.dma_start(cTi[a][:, s * 512:(s + 1) * 512],
                                      cT_d[a * 128:(a + 1) * 128,
                                           s * 512:(s + 1) * 512])
            nc.sync.dma_start(wo_sb[:], wo_d[:, :])
            for a in range(NT):
                nc.sync.dma_start(qTi[a][:, QE:lq],
                                  qT_d[a * 128:(a + 1) * 128, QE:lq])

            def wk_ap(a, t):
                return wkq_sb[:, a * 512 + t * 128:a * 512 + (t + 1) * 128]

            def wq_ap(a, t):
                return wkq_sb[:, a * 512 + 256 + t * 128:
                              a * 512 + 256 + (t + 1) * 128]

            def emit_all(reps_i):
                # producer gens (qproj/kproj) — must complete before their
                # consumer block's scores are EMITTED (in-order PE queue).
                prod = deque()
                # consumer gens (outproj) — their LDWEIGHTS reads attn, so
                # drain them only well after the normalize that writes it.
                late = deque()

                # Credit-based, GEN-ATOMIC drains: once a generator starts,
                # run it to completion (credit may go negative and is repaid
                # by later pumps).  A generator left mid-flight holds an sml
                # PSUM slot; any other sml allocation would then reuse that
                # slot and wait on the held gen's later-emitted reads — a
                # head-of-line deadlock in the in-order PE queue (v3.2 bug:
                # NRT_EXEC_UNIT_UNRECOVERABLE).
                credit = {"prod": 0, "late": 0}

                def drain(q, key, n):
                    if not q:
                        credit[key] = 0
                        return
                    credit[key] += n
                    while credit[key] > 0 and q:
                        g = q[0]
                        try:
                            while True:
                                next(g)
                                credit[key] -= 1
                        except StopIteration:
                            q.popleft()

                def qproj(t, e):
                    """q-projection for (d-tile t, q-block e), inline."""
                    ps = sml.tile([128, QE], F32, name="sml_ps", tag="sml")
                    for a in range(NT):
                        nc.tensor.matmul(
                            ps[:], wq_ap(a, t),
                            qTi[a][:, e * QE:(e + 1) * QE],
                            start=(a == 0), stop=(a == NT - 1))
                    nc.vector.tensor_scalar_add(
                        qTs[t][e][:], ps[:], bqk_sb[:, t:t + 1])

                def gen_qproj(t, e):
                    ps = sml.tile([128, QE], F32, name="sml_ps", tag="sml")
                    for a in range(NT):
                        nc.tensor.matmul(
                            ps[:], wq_ap(a, t),
                            qTi[a][:, e * QE:(e + 1) * QE],
                            start=(a == 0), stop=(a == NT - 1))
                        yield
                    nc.vector.tensor_scalar_add(
                        qTs[t][e][:], ps[:], bqk_sb[:, t:t + 1])

                def kproj_slab(t, qb):
                    """k-projection d-tile t for 512-col slab qb, inline."""
                    ps = sml.tile([128, 512], F32, name="sml_ps", tag="sml")
                    for a in range(NT):
                        nc.tensor.matmul(
                            ps[:], wk_ap(a, t),
                            cTi[a][:, qb * 512:(qb + 1) * 512],
                            start=(a == 0), stop=(a == NT - 1))
                    nc.vector.tensor_scalar_add(
                        kT[t][:, qb * 512:(qb + 1) * 512], ps[:],
                        bqk_sb[:, DT + t:DT + t + 1])

                def gen_kproj_slab(t, qb):
                    ps = sml.tile([128, 512], F32, name="sml_ps", tag="sml")
                    for a in range(NT):
                        nc.tensor.matmul(
                            ps[:], wk_ap(a, t),
                            cTi[a][:, qb * 512:(qb + 1) * 512],
                            start=(a == 0), stop=(a == NT - 1))
                        yield
                    nc.vector.tensor_scalar_add(
                        kT[t][:, qb * 512:(qb + 1) * 512], ps[:],
                        bqk_sb[:, DT + t:DT + t + 1])

                def vproj_half(ct, hp):
                    """v-projection for context chunk ct, head pair hp."""
                    HHD = 128  # two heads * 64 d
                    ps = sml.tile([128, 512], F32, name="sml_ps", tag="sml")
                    for a in range(NT):
                        nc.tensor.matmul(
                            ps[:, 0:HHD], cTi[a][:, ct * 128:(ct + 1) * 128],
                            wv_sb[:, a * HD + hp * HHD:
                                  a * HD + (hp + 1) * HHD],
                            start=(a == 0), stop=False)
                    nc.tensor.matmul(
                        ps[:, 0:HHD], ones_c[:, ct * 128:(ct + 1) * 128],
                        wvb_sb[:, hp * HHD:(hp + 1) * HHD],
                        start=False, stop=True)
                    vg = v_sb[ct].rearrange("p (g x) -> p g x", x=D + 1)
                    nc.vector.tensor_copy(
                        vg[:, 2 * hp:2 * hp + 2, 0:D],
                        ps[:, 0:HHD].rearrange("p (g x) -> p g x", x=D))
                    nc.vector.memset(vg[:, 2 * hp:2 * hp + 2, D:D + 1], 1.0)

                def gen_outproj(e, qt):
                    """out-projection for query tile qt of q-block e."""
                    ostg = osb.tile([128, MODEL], BF16, name="ostg", tag="ostg")
                    for nh in range(MODEL // 512):
                        ps = sml.tile([128, 512], F32, name="sml_ps",
                                      tag="sml")
                        for t in range(DT):
                            nc.tensor.matmul(
                                ps[:], attn[t][e][:, qt * 128:(qt + 1) * 128],
                                wo_sb[:, t * MODEL + nh * 512:
                                      t * MODEL + (nh + 1) * 512],
                                start=(t == 0), stop=(t == DT - 1))
                            yield
                        nc.vector.tensor_copy(
                            ostg[:, nh * 512:(nh + 1) * 512], ps[:])
                    nc.sync.dma_start(
                        out_d[(e * QE + qt * 128):(e * QE + (qt + 1) * 128), :],
                        ostg[:])

                # ---------- ramp: k-proj t0 slab 0 first (gates stream),
                # then q-proj (t0, e0), then remaining k-proj t0 slabs.
                kproj_slab(0, 0)
                qproj(0, 0)
                for qb in range(1, CS):
                    kproj_slab(0, qb)

                # ---------- producer queue (drained in PE slack) ----------
                # hp0 phase consumers: qTs[0][e] (e>=1), kT[1], qTs[1][*]
                for e in range(1, NQE):
                    prod.append(gen_qproj(0, e))
                for qb in range(CS):
                    prod.append(gen_kproj_slab(1, qb))
                prod.append(gen_qproj(1, 0))
                for e in range(1, NQE):
                    prod.append(gen_qproj(1, e))

                def norm_step(hp, e, asb, h, c):
                    """One normalize micro-step: 128-col chunk c of head h.
                    Chunked so (a) the scheduler's ~12x-optimistic reciprocal
                    cost model can only misplace ~1us of dependent work, and
                    (b) each outproj qt tile depends ONLY on its own chunk."""
                    f0, f1 = c * 128, (c + 1) * 128
                    def step():
                        rz = nrm.tile([1, 128], BF16, name="rz", tag="rz")
                        with nc.allow_low_precision(
                                reason="1/Z in bf16 (~0.4%), inside the "
                                       "2e-2 budget"):
                            nc.vector.reciprocal(rz[:], asb[64:65, f0:f1])
                        rb = nrm.tile([64, 128], BF16, name="rb", tag="rb")
                        nc.gpsimd.partition_broadcast(rb[:], rz[:])
                        if h == 0:
                            nc.vector.tensor_mul(
                                attn[hp][e][0:64, f0:f1], asb[0:64, f0:f1],
                                rb[:])
                        else:
                            at = atp.tile([64, 128], BF16, name="at",
                                          tag="at")
                            nc.vector.tensor_mul(at[:], asb[0:64, f0:f1],
                                                 rb[:])
                            nc.sync.dma_start(attn[hp][e][64:128, f0:f1],
                                              at[:])
                    return step

                norm_q = deque()   # pending normalize micro-steps (FIFO)

                # ---------- attention stream: hp outer, q-blocks inner ----
                for hp in range(DT):
                    for e in range(NQE):
                        av = [avp.tile([65, QE], F32, name=f"av{h}",
                                       tag=f"av{h}") for h in range(2)]
                        pend = deque()   # (pt, ct) awaiting AV emission

                        def emit_av(hp=hp, av=av, pend=pend):
                            ppt, pct = pend.popleft()
                            for h in range(2):
                                nc.tensor.matmul(
                                    av[h][:],
                                    v_sb[pct][:, (2 * hp + h) * (D + 1):
                                              (2 * hp + h + 1) * (D + 1)],
                                    ppt[:, h * QE:(h + 1) * QE],
                                    start=(pct == 0), stop=(pct == CT - 1))

                        for ct in range(CT):
                            # deferred normalize micro-steps, 2/ct once the
                            # boundary-critical DVE work is behind us (not
                            # in e==0 blocks: vproj owns the DVE there)
                            if e > 0 and ct >= 2:
                                for _ in range(2):
                                    if norm_q:
                                        norm_q.popleft()()
                            st = stp.tile([128, 2 * QE], F32, name="st",
                                          tag="st")
                            nc.tensor.matmul(
                                st[:, 0:QE],
                                kT[hp][0:64, ct * 128:(ct + 1) * 128],
                                qTs[hp][e][0:64, :],
                                start=True, stop=True)
                            nc.tensor.matmul(
                                st[:, QE:2 * QE],
                                kT[hp][64:128, ct * 128:(ct + 1) * 128],
                                qTs[hp][e][64:128, :],
                                start=True, stop=True)
                            pt = ptp.tile([128, 2 * QE], BF16, name="pt",
                                          tag="pt")
                            nc.scalar.activation(pt[:], st[:], EXPF,
                                                 scale=SCALE)
                            # JIT v-projection inside the first q-block of
                            # each hp phase (AV(ct) trails by 4 steps).
                            if e == 0:
                                vproj_half(ct, hp)
                            pend.append((pt, ct))
                            if len(pend) > 4:
                                emit_av()
                            if e > 0:
                                if ct < 8:
                                    drain(prod, "prod", 2)
                                elif ct >= 14:
                                    drain(late, "late", 5)
                        while pend:
                            emit_av()
                        # ---- normalize pair: copies NOW (free the PSUM av
                        # banks in ~1.2us); recip/bcast/mul deferred as
                        # chunked micro-steps, interleaved h0/h1 per chunk
                        # so outproj qt tiles unblock in qt order.
                        asbs = []
                        for h in range(2):
                            asb = avs.tile([65, QE], BF16, name="asb",
                                           tag="asb")
                            nc.vector.tensor_copy(asb[:], av[h][:])
                            asbs.append(asb)
                        for c in range(QE // 128):
                            for h in range(2):
                                norm_q.append(norm_step(hp, e, asbs[h], h, c))
                        # Boundary filler: outproj gens queued a block ago
                        # are fully ready (their norm finished mid-block) —
                        # they keep the PE busy while the last exps drain.
                        drain(late, "late", 10)
                        # Block-end drain: the next block's qTs/kT producers
                        # MUST complete before its first scores are emitted
                        # (the PE queue is in-order; a later-queued producer
                        # would deadlock an earlier-queued consumer).
                        drain(prod, "prod", 10)
                        # after pair 1 of block e: queue its out-projection
                        if hp == DT - 1:
                            for qt in range(QE // 128):
                                late.append(gen_outproj(e, qt))

                # ---------- tail ----------
                while norm_q:
                    norm_q.popleft()()
                drain(late, "late", 10 ** 9)
                drain(prod, "prod", 10 ** 9)

            for r in range(reps):
                emit_all(r)

    nc.compile()
    return nc


def _prep_inputs(query, context, Wq, bq, Wkv, bkv, Wout, lq, lc):
    """Host-side shard/cast/pack. Returns in_maps for 8 cores."""
    NT = MODEL // 128
    DT = HD // 128
    Wkv_r = np.asarray(Wkv, np.float32).reshape(MODEL, H, D, 2)
    bkv_r = np.asarray(bkv, np.float32).reshape(H, D, 2)
    Wq = np.asarray(Wq, np.float32)
    bq = np.asarray(bq, np.float32)
    Wout = np.asarray(Wout, np.float32)

    qT = [np.ascontiguousarray(np.asarray(query[b], np.float32).T).astype(NPBF16)
          for b in range(B)]
    cT = [np.ascontiguousarray(np.asarray(context[b], np.float32).T).astype(NPBF16)
          for b in range(B)]

    grp = []
    for g in range(GROUPS):
        hs = slice(g * HPC, (g + 1) * HPC)
        wq_c = Wq[:, g * HD:(g + 1) * HD]                      # [M, HD]
        wk_c = Wkv_r[:, hs, :, 0].reshape(MODEL, HD)           # [M, HD]
        wv_c = Wkv_r[:, hs, :, 1].reshape(MODEL, HD)           # [M, HD]
        bv_c = bkv_r[hs, :, 1].reshape(1, HD)
        wo_c = Wout[g * HD:(g + 1) * HD, :]                    # [HD, M]
        bq_c = bq[g * HD:(g + 1) * HD]
        bk_c = bkv_r[hs, :, 0].reshape(HD)

        # wkq packed [128, NT*512]: chunk a -> [wk_a | wq_a]
        wkq = np.empty((128, NT * 512), np.float32)
        for a in range(NT):
            wkq[:, a * 512:a * 512 + 256] = wk_c[a * 128:(a + 1) * 128, :]
            wkq[:, a * 512 + 256:(a + 1) * 512] = wq_c[a * 128:(a + 1) * 128, :]
        # wv packed [128, NT*HD]: chunk a at cols a*HD
        wvp = np.empty((128, NT * HD), np.float32)
        for a in range(NT):
            wvp[:, a * HD:(a + 1) * HD] = wv_c[a * 128:(a + 1) * 128, :]
        # wo packed [128, DT*MODEL]: d-tile t at cols t*MODEL
        wop = np.empty((128, DT * MODEL), np.float32)
        for t in range(DT):
            wop[:, t * MODEL:(t + 1) * MODEL] = wo_c[t * 128:(t + 1) * 128, :]
        # biases [128, 2*DT]: cols [bq_t0, bq_t1, bk_t0, bk_t1]
        bqk = np.empty((128, 2 * DT), np.float32)
        for t in range(DT):
            bqk[:, t] = bq_c[t * 128:(t + 1) * 128]
            bqk[:, DT + t] = bk_c[t * 128:(t + 1) * 128]
        grp.append((wkq.astype(NPBF16), wvp.astype(NPBF16),
                    np.ascontiguousarray(bv_c, np.float32).astype(NPBF16),
                    wop.astype(NPBF16), bqk))

    in_maps = []
    for c in range(N_CORES):
        b, g = c // GROUPS, c % GROUPS
        wkq, wvp, wvb, wop, bqk = grp[g]
        in_maps.append({
            "qT": qT[b], "cT": cT[b],
            "wkq": wkq, "wv": wvp, "wvb": wvb, "wo": wop, "bqk": bqk,
        })
    return in_maps


def _reduce_out(results, bout, lq):
    bout = np.asarray(bout, np.float32)
    out = np.empty((B, lq, MODEL), np.float32)
    for b in range(B):
        acc = results[b * GROUPS]["outp"].astype(np.float32)
        for g in range(1, GROUPS):
            acc += results[b * GROUPS + g]["outp"].astype(np.float32)
        out[b] = acc + bout
    return out


class Runner:
    """Cached-jit PJRT executor for the SPMD bass kernel (axon path).

    Mirrors bass2jax.run_bass_via_pjrt's multi-core branch, but builds the
    jitted sharded callable once so repeated calls reuse the compiled
    executable (for steady-state timing) and inputs can be pre-placed on
    device.
    """

    def __init__(self, nc, n_cores=N_CORES):
        import jax
        from jax.sharding import Mesh, PartitionSpec, NamedSharding
        from jax.experimental.shard_map import shard_map

        bass2jax.install_neuronx_cc_hook()
        assert nc.dbg_addr is None
        part_name = (nc.partition_id_tensor.name
                     if nc.partition_id_tensor else None)

        in_names, out_names, out_avals, zero_outs = [], [], [], []
        for alloc in nc.m.functions[0].allocations:
            if not isinstance(alloc, mybir.MemoryLocationSet):
                continue
            name = alloc.memorylocations[0].name
            if alloc.kind == "ExternalInput":
                if name != part_name:
                    in_names.append(name)
            elif alloc.kind == "ExternalOutput":
                shape = tuple(alloc.tensor_shape)
                dtype = mybir.dt.np(alloc.dtype)
                out_names.append(name)
                out_avals.append(jax.core.ShapedArray(shape, dtype))
                zero_outs.append(np.zeros(shape, dtype))
        self.n_params = len(in_names)
        self.in_names = list(in_names)
        self.out_names = out_names
        self.out_avals = out_avals
        self.zero_outs = zero_outs
        all_names = tuple(
            in_names + out_names + ([part_name] if part_name else []))

        def _body(*args):
            operands = list(args)
            if part_name is not None:
                operands.append(bass2jax.partition_id_tensor())
            outs = bass2jax._bass_exec_p.bind(
                *operands,
                out_avals=tuple(out_avals),
                in_names=all_names,
                out_names=tuple(out_names),
                lowering_input_output_aliases=(),
                sim_require_finite=True,
                sim_require_nnan=True,
                nc=nc,
            )
            return tuple(outs)

        devices = jax.devices()[:n_cores]
        assert len(devices) == n_cores
        self.n_cores = n_cores
        self.mesh = Mesh(np.asarray(devices), ("core",))
        self.sharding = NamedSharding(self.mesh, PartitionSpec("core"))
        n_args = self.n_params + len(out_names)
        self.fn = jax.jit(
            shard_map(_body, mesh=self.mesh,
                      in_specs=(PartitionSpec("core"),) * n_args,
                      out_specs=(PartitionSpec("core"),) * len(out_names),
                      check_rep=False),
            keep_unused=True,
        )
        self._jax = jax

    def put(self, in_maps):
        """Concatenate per-core inputs on axis 0 and place on devices."""
        jax = self._jax
        args = []
        for name in self.in_names:
            arr = np.concatenate(
                [np.asarray(m[name]) for m in in_maps], axis=0)
            args.append(jax.device_put(arr, self.sharding))
        for z in self.zero_outs:
            zz = np.zeros((self.n_cores * z.shape[0], *z.shape[1:]), z.dtype)
            args.append(jax.device_put(zz, self.sharding))
        return args

    def call(self, dev_args):
        outs = self.fn(*dev_args)
        self._jax.block_until_ready(outs)
        return outs

    def gather(self, outs):
        """outs -> list (per core) of {name: np.ndarray}."""
        res = []
        for c in range(self.n_cores):
            d = {}
            for i, name in enumerate(self.out_names):
                full = np.asarray(outs[i])
                d[name] = full.reshape(
                    self.n_cores, *self.out_avals[i].shape)[c]
            res.append(d)
        return res


_CACHE = {}


def _get_runner(lq, lc):
    key = (lq, lc)
    if key not in _CACHE:
        _CACHE[key] = Runner(build_nc(lq, lc))
    return _CACHE[key]


def run(query, context, Wq, bq, Wkv, bkv, Wout, bout):
    lq, lc = query.shape[1], context.shape[1]
    runner = _get_runner(lq, lc)
    in_maps = _prep_inputs(query, context, Wq, bq, Wkv, bkv, Wout, lq, lc)
    dev_args = runner.put(in_maps)
    outs = runner.call(dev_args)
    results = runner.gather(outs)
    return _reduce_out(results, bout, lq), runner, dev_args


def kernel(query, context, Wq, bq, Wkv, bkv, Wout, bout):
    out, _, _ = run(query, context, Wq, bq, Wkv, bkv, Wout, bout)
    return out


# revision 31
# speedup vs baseline: 2.2368x; 1.0108x over previous
"""CrossAttention Trainium2 kernel, v3 (PE-tiling + HAM-warm restructure).

Full inputs in, full output out. Sharding: data-parallel over batch (B=2),
tensor-parallel over heads (16 heads -> 4 groups of 4), 8 cores total.
Each core computes attention for its (batch, 4-head group) and a partial
output projection in bf16; the host sums the 4 partials per batch and adds
bout.

v3 changes vs v2 (trace-driven):
  - Head-PAIR score matmuls: the two heads of a kT/qTs tile live on PE
    row-groups 0 and 64 (K=64 each); issuing them back-to-back runs them
    concurrently on the PE (row tiling), halving score time.
  - QE=512 q-blocks; the pair's scores land in one [128, 1024] PSUM tile
    (2 banks) -> ONE exp instruction per (pair, ct) keeps the ACT
    instruction count at 128 while PSUM fits (st 4 + av 2 + sml 2 banks).
  - Normalization off the PE critical path: av [65, 512] PSUM tiles are
    copied to SBUF immediately (frees the bank in ~0.6us), then
    reciprocal/broadcast/multiply run concurrently with the next pair's
    stream.  v2 held av through a 5.8us single-lane reciprocal, stalling
    the PE ~6us at every head boundary and re-throttling HAM to half
    clock (148us of K=4/8 measured).
  - hp-outer loop (all q-blocks for head-pair 0, then pair 1): kT[1]
    projection + out-projections ride later slack; v-projection is split
    per head-pair and JIT'd inside the e0 passes.
  - Column-slab cT DMAs so k-projection (and the stream) starts after the
    first 0.5MB slab instead of the full 4MB.
"""

import numpy as np
import ml_dtypes
from collections import deque

import concourse.bass as bass
import concourse.tile as tile
from concourse import bacc, mybir
from concourse import bass2jax

BF16 = mybir.dt.bfloat16
F32 = mybir.dt.float32
NPBF16 = ml_dtypes.bfloat16

B, MODEL, H, D = 2, 1024, 16, 64
LQ, LC = 2048, 2048
N_CORES = 8
GROUPS = 4            # head groups = cores per batch
HPC = H // GROUPS     # heads per core (4)
HD = HPC * D          # 256 head-dims per core
SCALE = D ** -0.5
EXPF = mybir.ActivationFunctionType.Exp


def build_nc(lq=LQ, lc=LC, reps=1):
    NT = MODEL // 128         # m-contraction chunks (8)
    DT = HD // 128            # d-tiles / head-pairs (2)
    CT = lc // 128            # context chunks (16)
    QE = 512                  # q-block (free dim of scores)
    NQE = lq // QE            # 4
    CS = lc // 512            # 512-col context slabs (4)

    nc = bacc.Bacc(trn_type="TRN2", target_bir_lowering=False, debug=False,
                   num_devices=N_CORES)

    cT_d = nc.dram_tensor("cT", [MODEL, lc], BF16, kind="ExternalInput").ap()
    qT_d = nc.dram_tensor("qT", [MODEL, lq], BF16, kind="ExternalInput").ap()
    # packed: per chunk a: cols [a*512 : a*512+256] = wk_a, [+256 : +512] = wq_a
    wkq_d = nc.dram_tensor("wkq", [128, NT * 512], BF16,
                           kind="ExternalInput").ap()
    wv_d = nc.dram_tensor("wv", [128, NT * HD], BF16, kind="ExternalInput").ap()
    wvb_d = nc.dram_tensor("wvb", [1, HD], BF16, kind="ExternalInput").ap()
    wo_d = nc.dram_tensor("wo", [128, DT * MODEL], BF16,
                          kind="ExternalInput").ap()
    # cols: [bq_t0, bq_t1, bk_t0, bk_t1]
    bqk_d = nc.dram_tensor("bqk", [128, 2 * DT], F32, kind="ExternalInput").ap()
    out_d = nc.dram_tensor("outp", [lq, MODEL], BF16, kind="ExternalOutput").ap()

    with tile.TileContext(nc) as tc:
        with tc.tile_pool(name="w", bufs=1) as wp, \
             tc.tile_pool(name="acts", bufs=1) as acp, \
             tc.tile_pool(name="ptp", bufs=7) as ptp, \
             tc.tile_pool(name="avs", bufs=6) as avs, \
             tc.tile_pool(name="nrm", bufs=4) as nrm, \
             tc.tile_pool(name="atp", bufs=4) as atp, \
             tc.tile_pool(name="osb", bufs=3) as osb, \
             tc.tile_pool(name="stp", bufs=2, space="PSUM") as stp, \
             tc.tile_pool(name="avp", bufs=1, space="PSUM") as avp, \
             tc.tile_pool(name="sml", bufs=2, space="PSUM") as sml:

            # ---- activation-table warmup (hide LoadActFuncSet in the ramp)
            warm_in = acp.tile([1, 8], F32, name="warm_in", tag="warm_in")
            warm_out = acp.tile([1, 8], BF16, name="warm_out", tag="warm_out")
            nc.vector.memset(warm_in[:], 0.0)
            nc.scalar.activation(warm_out[:], warm_in[:], EXPF, scale=SCALE)

            # ---- SBUF tiles
            wkq_sb = wp.tile([128, NT * 512], BF16, name="wkq", tag="wkq")
            wv_sb = wp.tile([128, NT * HD], BF16, name="wv", tag="wv")
            wvb_sb = wp.tile([1, HD], BF16, name="wvb", tag="wvb")
            wo_sb = wp.tile([128, DT * MODEL], BF16, name="wo", tag="wo")
            bqk_sb = wp.tile([128, 2 * DT], F32, name="bqk", tag="bqk")
            cTi = [acp.tile([128, lc], BF16, name=f"cTi{a}", tag=f"cTi{a}")
                   for a in range(NT)]
            qTi = [acp.tile([128, lq], BF16, name=f"qTi{a}", tag=f"qTi{a}")
                   for a in range(NT)]
            ones_c = acp.tile([1, lc], BF16, name="ones_c", tag="ones_c")
            nc.vector.memset(ones_c[:], 1.0)

            kT = [acp.tile([128, lc], BF16, name=f"kT{t}", tag=f"kT{t}")
                  for t in range(DT)]
            qTs = [[acp.tile([128, QE], BF16, name=f"qTs{t}_{e}",
                             tag=f"qTs{t}_{e}") for e in range(NQE)]
                   for t in range(DT)]
            v_sb = [acp.tile([128, HPC * (D + 1)], BF16, name=f"v{ct}",
                             tag=f"v{ct}") for ct in range(CT)]
            attn = [[acp.tile([128, QE], BF16, name=f"attn{t}_{e}",
                              tag=f"attn{t}_{e}") for e in range(NQE)]
                    for t in range(DT)]

            # ---- input DMAs, priority order (arrival order = issue order).
            # wkq in chunks so kproj slab 0 starts after the first 128KB;
            # cT slab 0 ahead of qT/wv: kproj slab 0 gates the stream start.
            for a in range(NT):
                nc.sync.dma_start(wkq_sb[:, a * 512:(a + 1) * 512],
                                  wkq_d[:, a * 512:(a + 1) * 512])
            nc.sync.dma_start(bqk_sb[:], bqk_d[:, :])
            for a in range(NT):
                nc.sync.dma_start(cTi[a][:, 0:512],
                                  cT_d[a * 128:(a + 1) * 128, 0:512])
            for a in range(NT):
                nc.sync.dma_start(qTi[a][:, 0:QE], qT_d[a * 128:(a + 1) * 128,
                                                        0:QE])
            nc.sync.dma_start(wv_sb[:], wv_d[:, :])
            nc.sync.dma_start(wvb_sb[:], wvb_d[:, :])
            # remaining cT in 512-col slabs so kproj/vproj chase arrivals.
            for s in range(1, CS):
                for a in range(NT):
                    nc.sync.dma_start(cTi[a][:, s * 512:(s + 1) * 512],
                                      cT_d[a * 128:(a + 1) * 128,
                                           s * 512:(s + 1) * 512])
            nc.sync.dma_start(wo_sb[:], wo_d[:, :])
            for a in range(NT):
                nc.sync.dma_start(qTi[a][:, QE:lq],
                                  qT_d[a * 128:(a + 1) * 128, QE:lq])

            def wk_ap(a, t):
                return wkq_sb[:, a * 512 + t * 128:a * 512 + (t + 1) * 128]

            def wq_ap(a, t):
                return wkq_sb[:, a * 512 + 256 + t * 128:
                              a * 512 + 256 + (t + 1) * 128]

            def emit_all(reps_i):
                # producer gens (qproj/kproj) — must complete before their
                # consumer block's scores are EMITTED (in-order PE queue).
                prod = deque()
                # consumer gens (outproj) — their LDWEIGHTS reads attn, so
                # drain them only well after the normalize that writes it.
                late = deque()

                # Credit-based, GEN-ATOMIC drains: once a generator starts,
                # run it to completion (credit may go negative and is repaid
                # by later pumps).  A generator left mid-flight holds an sml
                # PSUM slot; any other sml allocation would then reuse that
                # slot and wait on the held gen's later-emitted reads — a
                # head-of-line deadlock in the in-order PE queue (v3.2 bug:
                # NRT_EXEC_UNIT_UNRECOVERABLE).
                credit = {"prod": 0, "late": 0}

                def drain(q, key, n):
                    if not q:
                        credit[key] = 0
                        return
                    credit[key] += n
                    while credit[key] > 0 and q:
                        g = q[0]
                        try:
                            while True:
                                next(g)
                                credit[key] -= 1
                        except StopIteration:
                            q.popleft()

                def qproj(t, e):
                    """q-projection for (d-tile t, q-block e), inline."""
                    ps = sml.tile([128, QE], F32, name="sml_ps", tag="sml")
                    for a in range(NT):
                        nc.tensor.matmul(
                            ps[:], wq_ap(a, t),
                            qTi[a][:, e * QE:(e + 1) * QE],
                            start=(a == 0), stop=(a == NT - 1))
                    nc.vector.tensor_scalar_add(
                        qTs[t][e][:], ps[:], bqk_sb[:, t:t + 1])

                def gen_qproj(t, e):
                    ps = sml.tile([128, QE], F32, name="sml_ps", tag="sml")
                    for a in range(NT):
                        nc.tensor.matmul(
                            ps[:], wq_ap(a, t),
                            qTi[a][:, e * QE:(e + 1) * QE],
                            start=(a == 0), stop=(a == NT - 1))
                        yield
                    nc.vector.tensor_scalar_add(
                        qTs[t][e][:], ps[:], bqk_sb[:, t:t + 1])

                def kproj_slab(t, qb):
                    """k-projection d-tile t for 512-col slab qb, inline."""
                    ps = sml.tile([128, 512], F32, name="sml_ps", tag="sml")
                    for a in range(NT):
                        nc.tensor.matmul(
                            ps[:], wk_ap(a, t),
                            cTi[a][:, qb * 512:(qb + 1) * 512],
                            start=(a == 0), stop=(a == NT - 1))
                    nc.vector.tensor_scalar_add(
                        kT[t][:, qb * 512:(qb + 1) * 512], ps[:],
                        bqk_sb[:, DT + t:DT + t + 1])

                def gen_kproj_slab(t, qb):
                    ps = sml.tile([128, 512], F32, name="sml_ps", tag="sml")
                    for a in range(NT):
                        nc.tensor.matmul(
                            ps[:], wk_ap(a, t),
                            cTi[a][:, qb * 512:(qb + 1) * 512],
                            start=(a == 0), stop=(a == NT - 1))
                        yield
                    nc.vector.tensor_scalar_add(
                        kT[t][:, qb * 512:(qb + 1) * 512], ps[:],
                        bqk_sb[:, DT + t:DT + t + 1])

                def vproj_half(ct, hp):
                    """v-projection for context chunk ct, head pair hp."""
                    HHD = 128  # two heads * 64 d
                    ps = sml.tile([128, 512], F32, name="sml_ps", tag="sml")
                    for a in range(NT):
                        nc.tensor.matmul(
                            ps[:, 0:HHD], cTi[a][:, ct * 128:(ct + 1) * 128],
                            wv_sb[:, a * HD + hp * HHD:
                                  a * HD + (hp + 1) * HHD],
                            start=(a == 0), stop=False)
                    nc.tensor.matmul(
                        ps[:, 0:HHD], ones_c[:, ct * 128:(ct + 1) * 128],
                        wvb_sb[:, hp * HHD:(hp + 1) * HHD],
                        start=False, stop=True)
                    vg = v_sb[ct].rearrange("p (g x) -> p g x", x=D + 1)
                    nc.vector.tensor_copy(
                        vg[:, 2 * hp:2 * hp + 2, 0:D],
                        ps[:, 0:HHD].rearrange("p (g x) -> p g x", x=D))
                    nc.vector.memset(vg[:, 2 * hp:2 * hp + 2, D:D + 1], 1.0)

                def gen_outproj(e, qt):
                    """out-projection for query tile qt of q-block e."""
                    ostg = osb.tile([128, MODEL], BF16, name="ostg", tag="ostg")
                    for nh in range(MODEL // 512):
                        ps = sml.tile([128, 512], F32, name="sml_ps",
                                      tag="sml")
                        for t in range(DT):
                            nc.tensor.matmul(
                                ps[:], attn[t][e][:, qt * 128:(qt + 1) * 128],
                                wo_sb[:, t * MODEL + nh * 512:
                                      t * MODEL + (nh + 1) * 512],
                                start=(t == 0), stop=(t == DT - 1))
                            yield
                        nc.vector.tensor_copy(
                            ostg[:, nh * 512:(nh + 1) * 512], ps[:])
                    nc.sync.dma_start(
                        out_d[(e * QE + qt * 128):(e * QE + (qt + 1) * 128), :],
                        ostg[:])

                # ---------- ramp: k-proj t0 slab 0 first (gates stream),
                # then q-proj (t0, e0), then remaining k-proj t0 slabs.
                kproj_slab(0, 0)
                qproj(0, 0)
                for qb in range(1, CS):
                    kproj_slab(0, qb)

                # ---------- producer queue (drained in PE slack) ----------
                # hp0 phase consumers: qTs[0][e] (e>=1), kT[1], qTs[1][*]
                for e in range(1, NQE):
                    prod.append(gen_qproj(0, e))
                for qb in range(CS):
                    prod.append(gen_kproj_slab(1, qb))
                prod.append(gen_qproj(1, 0))
                for e in range(1, NQE):
                    prod.append(gen_qproj(1, e))

                def norm_step(hp, e, asb, h, c):
                    """One normalize micro-step: 128-col chunk c of head h.
                    Chunked so (a) the scheduler's ~12x-optimistic reciprocal
                    cost model can only misplace ~1us of dependent work, and
                    (b) each outproj qt tile depends ONLY on its own chunk."""
                    f0, f1 = c * 128, (c + 1) * 128
                    def step():
                        rz = nrm.tile([1, 128], BF16, name="rz", tag="rz")
                        with nc.allow_low_precision(
                                reason="1/Z in bf16 (~0.4%), inside the "
                                       "2e-2 budget"):
                            nc.vector.reciprocal(rz[:], asb[64:65, f0:f1])
                        rb = nrm.tile([64, 128], BF16, name="rb", tag="rb")
                        nc.gpsimd.partition_broadcast(rb[:], rz[:])
                        if h == 0:
                            nc.vector.tensor_mul(
                                attn[hp][e][0:64, f0:f1], asb[0:64, f0:f1],
                                rb[:])
                        else:
                            at = atp.tile([64, 128], BF16, name="at",
                                          tag="at")
                            nc.vector.tensor_mul(at[:], asb[0:64, f0:f1],
                                                 rb[:])
                            nc.sync.dma_start(attn[hp][e][64:128, f0:f1],
                                              at[:])
                    return step

                norm_q = deque()   # pending normalize micro-steps (FIFO)

                # ---------- attention stream: hp outer, q-blocks inner ----
                for hp in range(DT):
                    for e in range(NQE):
                        av = [avp.tile([65, QE], F32, name=f"av{h}",
                                       tag=f"av{h}") for h in range(2)]
                        pend = deque()   # (pt, ct) awaiting AV emission

                        def emit_av(hp=hp, av=av, pend=pend):
                            ppt, pct = pend.popleft()
                            for h in range(2):
                                nc.tensor.matmul(
                                    av[h][:],
                                    v_sb[pct][:, (2 * hp + h) * (D + 1):
                                              (2 * hp + h + 1) * (D + 1)],
                                    ppt[:, h * QE:(h + 1) * QE],
                                    start=(pct == 0), stop=(pct == CT - 1))

                        for ct in range(CT):
                            # deferred normalize micro-steps, 2/ct once the
                            # boundary-critical DVE work is behind us (not
                            # in e==0 blocks: vproj owns the DVE there)
                            if e > 0 and ct >= 2:
                                for _ in range(2):
                                    if norm_q:
                                        norm_q.popleft()()
                            st = stp.tile([128, 2 * QE], F32, name="st",
                                          tag="st")
                            nc.tensor.matmul(
                                st[:, 0:QE],
                                kT[hp][0:64, ct * 128:(ct + 1) * 128],
                                qTs[hp][e][0:64, :],
                                start=True, stop=True)
                            nc.tensor.matmul(
                                st[:, QE:2 * QE],
                                kT[hp][64:128, ct * 128:(ct + 1) * 128],
                                qTs[hp][e][64:128, :],
                                start=True, stop=True)
                            pt = ptp.tile([128, 2 * QE], BF16, name="pt",
                                          tag="pt")
                            nc.scalar.activation(pt[:], st[:], EXPF,
                                                 scale=SCALE)
                            # JIT v-projection inside the first q-block of
                            # each hp phase (AV(ct) trails by 4 steps).
                            if e == 0:
                                vproj_half(ct, hp)
                            pend.append((pt, ct))
                            if len(pend) > 4:
                                emit_av()
                            if e > 0:
                                if ct < 8:
                                    drain(prod, "prod", 2)
                                elif ct >= 14:
                                    drain(late, "late", 5)
                        # Boundary fillers BEFORE the trailing-AV drain: the
                        # last AVs wait on the last exps (ACT-paced), so
                        # ready outproj/producer matmuls fill that bubble.
                        drain(late, "late", 10)
                        drain(prod, "prod", 10)
                        while pend:
                            emit_av()
                        # ---- normalize pair: copies NOW (free the PSUM av
                        # banks in ~1.2us); recip/bcast/mul deferred as
                        # chunked micro-steps, interleaved h0/h1 per chunk
                        # so outproj qt tiles unblock in qt order.
                        asbs = []
                        for h in range(2):
                            asb = avs.tile([65, QE], BF16, name="asb",
                                           tag="asb")
                            nc.vector.tensor_copy(asb[:], av[h][:])
                            asbs.append(asb)
                        for c in range(QE // 128):
                            for h in range(2):
                                norm_q.append(norm_step(hp, e, asbs[h], h, c))
                        # after pair 1 of block e: queue its out-projection
                        if hp == DT - 1:
                            for qt in range(QE // 128):
                                late.append(gen_outproj(e, qt))

                # ---------- tail ----------
                while norm_q:
                    norm_q.popleft()()
                drain(late, "late", 10 ** 9)
                drain(prod, "prod", 10 ** 9)

            for r in range(reps):
                emit_all(r)

    nc.compile()
    return nc


def _prep_inputs(query, context, Wq, bq, Wkv, bkv, Wout, lq, lc):
    """Host-side shard/cast/pack. Returns in_maps for 8 cores."""
    NT = MODEL // 128
    DT = HD // 128
    Wkv_r = np.asarray(Wkv, np.float32).reshape(MODEL, H, D, 2)
    bkv_r = np.asarray(bkv, np.float32).reshape(H, D, 2)
    Wq = np.asarray(Wq, np.float32)
    bq = np.asarray(bq, np.float32)
    Wout = np.asarray(Wout, np.float32)

    qT = [np.ascontiguousarray(np.asarray(query[b], np.float32).T).astype(NPBF16)
          for b in range(B)]
    cT = [np.ascontiguousarray(np.asarray(context[b], np.float32).T).astype(NPBF16)
          for b in range(B)]

    grp = []
    for g in range(GROUPS):
        hs = slice(g * HPC, (g + 1) * HPC)
        wq_c = Wq[:, g * HD:(g + 1) * HD]                      # [M, HD]
        wk_c = Wkv_r[:, hs, :, 0].reshape(MODEL, HD)           # [M, HD]
        wv_c = Wkv_r[:, hs, :, 1].reshape(MODEL, HD)           # [M, HD]
        bv_c = bkv_r[hs, :, 1].reshape(1, HD)
        wo_c = Wout[g * HD:(g + 1) * HD, :]                    # [HD, M]
        bq_c = bq[g * HD:(g + 1) * HD]
        bk_c = bkv_r[hs, :, 0].reshape(HD)

        # wkq packed [128, NT*512]: chunk a -> [wk_a | wq_a]
        wkq = np.empty((128, NT * 512), np.float32)
        for a in range(NT):
            wkq[:, a * 512:a * 512 + 256] = wk_c[a * 128:(a + 1) * 128, :]
            wkq[:, a * 512 + 256:(a + 1) * 512] = wq_c[a * 128:(a + 1) * 128, :]
        # wv packed [128, NT*HD]: chunk a at cols a*HD
        wvp = np.empty((128, NT * HD), np.float32)
        for a in range(NT):
            wvp[:, a * HD:(a + 1) * HD] = wv_c[a * 128:(a + 1) * 128, :]
        # wo packed [128, DT*MODEL]: d-tile t at cols t*MODEL
        wop = np.empty((128, DT * MODEL), np.float32)
        for t in range(DT):
            wop[:, t * MODEL:(t + 1) * MODEL] = wo_c[t * 128:(t + 1) * 128, :]
        # biases [128, 2*DT]: cols [bq_t0, bq_t1, bk_t0, bk_t1]
        bqk = np.empty((128, 2 * DT), np.float32)
        for t in range(DT):
            bqk[:, t] = bq_c[t * 128:(t + 1) * 128]
            bqk[:, DT + t] = bk_c[t * 128:(t + 1) * 128]
        grp.append((wkq.astype(NPBF16), wvp.astype(NPBF16),
                    np.ascontiguousarray(bv_c, np.float32).astype(NPBF16),
                    wop.astype(NPBF16), bqk))

    in_maps = []
    for c in range(N_CORES):
        b, g = c // GROUPS, c % GROUPS
        wkq, wvp, wvb, wop, bqk = grp[g]
        in_maps.append({
            "qT": qT[b], "cT": cT[b],
            "wkq": wkq, "wv": wvp, "wvb": wvb, "wo": wop, "bqk": bqk,
        })
    return in_maps


def _reduce_out(results, bout, lq):
    bout = np.asarray(bout, np.float32)
    out = np.empty((B, lq, MODEL), np.float32)
    for b in range(B):
        acc = results[b * GROUPS]["outp"].astype(np.float32)
        for g in range(1, GROUPS):
            acc += results[b * GROUPS + g]["outp"].astype(np.float32)
        out[b] = acc + bout
    return out


class Runner:
    """Cached-jit PJRT executor for the SPMD bass kernel (axon path).

    Mirrors bass2jax.run_bass_via_pjrt's multi-core branch, but builds the
    jitted sharded callable once so repeated calls reuse the compiled
    executable (for steady-state timing) and inputs can be pre-placed on
    device.
    """

    def __init__(self, nc, n_cores=N_CORES):
        import jax
        from jax.sharding import Mesh, PartitionSpec, NamedSharding
        from jax.experimental.shard_map import shard_map

        bass2jax.install_neuronx_cc_hook()
        assert nc.dbg_addr is None
        part_name = (nc.partition_id_tensor.name
                     if nc.partition_id_tensor else None)

        in_names, out_names, out_avals, zero_outs = [], [], [], []
        for alloc in nc.m.functions[0].allocations:
            if not isinstance(alloc, mybir.MemoryLocationSet):
                continue
            name = alloc.memorylocations[0].name
            if alloc.kind == "ExternalInput":
                if name != part_name:
                    in_names.append(name)
            elif alloc.kind == "ExternalOutput":
                shape = tuple(alloc.tensor_shape)
                dtype = mybir.dt.np(alloc.dtype)
                out_names.append(name)
                out_avals.append(jax.core.ShapedArray(shape, dtype))
                zero_outs.append(np.zeros(shape, dtype))
        self.n_params = len(in_names)
        self.in_names = list(in_names)
        self.out_names = out_names
        self.out_avals = out_avals
        self.zero_outs = zero_outs
        all_names = tuple(
            in_names + out_names + ([part_name] if part_name else []))

        def _body(*args):
            operands = list(args)
            if part_name is not None:
                operands.append(bass2jax.partition_id_tensor())
            outs = bass2jax._bass_exec_p.bind(
                *operands,
                out_avals=tuple(out_avals),
                in_names=all_names,
                out_names=tuple(out_names),
                lowering_input_output_aliases=(),
                sim_require_finite=True,
                sim_require_nnan=True,
                nc=nc,
            )
            return tuple(outs)

        devices = jax.devices()[:n_cores]
        assert len(devices) == n_cores
        self.n_cores = n_cores
        self.mesh = Mesh(np.asarray(devices), ("core",))
        self.sharding = NamedSharding(self.mesh, PartitionSpec("core"))
        n_args = self.n_params + len(out_names)
        self.fn = jax.jit(
            shard_map(_body, mesh=self.mesh,
                      in_specs=(PartitionSpec("core"),) * n_args,
                      out_specs=(PartitionSpec("core"),) * len(out_names),
                      check_rep=False),
            keep_unused=True,
        )
        self._jax = jax

    def put(self, in_maps):
        """Concatenate per-core inputs on axis 0 and place on devices."""
        jax = self._jax
        args = []
        for name in self.in_names:
            arr = np.concatenate(
                [np.asarray(m[name]) for m in in_maps], axis=0)
            args.append(jax.device_put(arr, self.sharding))
        for z in self.zero_outs:
            zz = np.zeros((self.n_cores * z.shape[0], *z.shape[1:]), z.dtype)
            args.append(jax.device_put(zz, self.sharding))
        return args

    def call(self, dev_args):
        outs = self.fn(*dev_args)
        self._jax.block_until_ready(outs)
        return outs

    def gather(self, outs):
        """outs -> list (per core) of {name: np.ndarray}."""
        res = []
        for c in range(self.n_cores):
            d = {}
            for i, name in enumerate(self.out_names):
                full = np.asarray(outs[i])
                d[name] = full.reshape(
                    self.n_cores, *self.out_avals[i].shape)[c]
            res.append(d)
        return res


_CACHE = {}


def _get_runner(lq, lc):
    key = (lq, lc)
    if key not in _CACHE:
        _CACHE[key] = Runner(build_nc(lq, lc))
    return _CACHE[key]


def run(query, context, Wq, bq, Wkv, bkv, Wout, bout):
    lq, lc = query.shape[1], context.shape[1]
    runner = _get_runner(lq, lc)
    in_maps = _prep_inputs(query, context, Wq, bq, Wkv, bkv, Wout, lq, lc)
    dev_args = runner.put(in_maps)
    outs = runner.call(dev_args)
    results = runner.gather(outs)
    return _reduce_out(results, bout, lq), runner, dev_args


def kernel(query, context, Wq, bq, Wkv, bkv, Wout, bout):
    out, _, _ = run(query, context, Wq, bq, Wkv, bkv, Wout, bout)
    return out


# revision 39
# speedup vs baseline: 2.2475x; 1.0048x over previous
"""CrossAttention Trainium2 kernel, v3 (PE-tiling + HAM-warm restructure).

Full inputs in, full output out. Sharding: data-parallel over batch (B=2),
tensor-parallel over heads (16 heads -> 4 groups of 4), 8 cores total.
Each core computes attention for its (batch, 4-head group) and a partial
output projection in bf16; the host sums the 4 partials per batch and adds
bout.

v3 changes vs v2 (trace-driven):
  - Head-PAIR score matmuls: the two heads of a kT/qTs tile live on PE
    row-groups 0 and 64 (K=64 each); issuing them back-to-back runs them
    concurrently on the PE (row tiling), halving score time.
  - QE=512 q-blocks; the pair's scores land in one [128, 1024] PSUM tile
    (2 banks) -> ONE exp instruction per (pair, ct) keeps the ACT
    instruction count at 128 while PSUM fits (st 4 + av 2 + sml 2 banks).
  - Normalization off the PE critical path: av [65, 512] PSUM tiles are
    copied to SBUF immediately (frees the bank in ~0.6us), then
    reciprocal/broadcast/multiply run concurrently with the next pair's
    stream.  v2 held av through a 5.8us single-lane reciprocal, stalling
    the PE ~6us at every head boundary and re-throttling HAM to half
    clock (148us of K=4/8 measured).
  - hp-outer loop (all q-blocks for head-pair 0, then pair 1): kT[1]
    projection + out-projections ride later slack; v-projection is split
    per head-pair and JIT'd inside the e0 passes.
  - Column-slab cT DMAs so k-projection (and the stream) starts after the
    first 0.5MB slab instead of the full 4MB.
"""

import numpy as np
import ml_dtypes
from collections import deque

import concourse.bass as bass
import concourse.tile as tile
from concourse import bacc, mybir
from concourse import bass2jax

BF16 = mybir.dt.bfloat16
F32 = mybir.dt.float32
NPBF16 = ml_dtypes.bfloat16

B, MODEL, H, D = 2, 1024, 16, 64
LQ, LC = 2048, 2048
N_CORES = 8
GROUPS = 4            # head groups = cores per batch
HPC = H // GROUPS     # heads per core (4)
HD = HPC * D          # 256 head-dims per core
SCALE = D ** -0.5
EXPF = mybir.ActivationFunctionType.Exp


def build_nc(lq=LQ, lc=LC, reps=1):
    NT = MODEL // 128         # m-contraction chunks (8)
    DT = HD // 128            # d-tiles / head-pairs (2)
    CT = lc // 128            # context chunks (16)
    QE = 512                  # q-block (free dim of scores)
    NQE = lq // QE            # 4
    CS = lc // 512            # 512-col context slabs (4)
    QE_LAST = QE              # rows of the out2 t1-partial (last q-block)

    nc = bacc.Bacc(trn_type="TRN2", target_bir_lowering=False, debug=False,
                   num_devices=N_CORES)

    cT_d = nc.dram_tensor("cT", [MODEL, lc], BF16, kind="ExternalInput").ap()
    qT_d = nc.dram_tensor("qT", [MODEL, lq], BF16, kind="ExternalInput").ap()
    # packed: per chunk a: cols [a*512 : a*512+256] = wk_a, [+256 : +512] = wq_a
    wkq_d = nc.dram_tensor("wkq", [128, NT * 512], BF16,
                           kind="ExternalInput").ap()
    wv_d = nc.dram_tensor("wv", [128, NT * HD], BF16, kind="ExternalInput").ap()
    wvb_d = nc.dram_tensor("wvb", [1, HD], BF16, kind="ExternalInput").ap()
    wo_d = nc.dram_tensor("wo", [128, DT * MODEL], BF16,
                          kind="ExternalInput").ap()
    # cols: [bq_t0, bq_t1, bk_t0, bk_t1]
    bqk_d = nc.dram_tensor("bqk", [128, 2 * DT], F32, kind="ExternalInput").ap()
    out_d = nc.dram_tensor("outp", [lq, MODEL], BF16, kind="ExternalOutput").ap()
    # t1-partial of the LAST q-block's out-projection (host adds it):
    # lets the t0 half run mid-stream instead of serializing in the tail.
    out2_d = nc.dram_tensor("outp2", [QE_LAST, MODEL], BF16,
                            kind="ExternalOutput").ap()

    with tile.TileContext(nc) as tc:
        with tc.tile_pool(name="w", bufs=1) as wp, \
             tc.tile_pool(name="acts", bufs=1) as acp, \
             tc.tile_pool(name="ptp", bufs=7) as ptp, \
             tc.tile_pool(name="avs", bufs=6) as avs, \
             tc.tile_pool(name="nrm", bufs=4) as nrm, \
             tc.tile_pool(name="atp", bufs=4) as atp, \
             tc.tile_pool(name="osb", bufs=3) as osb, \
             tc.tile_pool(name="stp", bufs=2, space="PSUM") as stp, \
             tc.tile_pool(name="avp", bufs=1, space="PSUM") as avp, \
             tc.tile_pool(name="sml", bufs=2, space="PSUM") as sml:

            # ---- activation-table warmup (hide LoadActFuncSet in the ramp)
            warm_in = acp.tile([1, 8], F32, name="warm_in", tag="warm_in")
            warm_out = acp.tile([1, 8], BF16, name="warm_out", tag="warm_out")
            nc.vector.memset(warm_in[:], 0.0)
            nc.scalar.activation(warm_out[:], warm_in[:], EXPF, scale=SCALE)

            # ---- SBUF tiles
            wkq_sb = wp.tile([128, NT * 512], BF16, name="wkq", tag="wkq")
            wv_sb = wp.tile([128, NT * HD], BF16, name="wv", tag="wv")
            wvb_sb = wp.tile([1, HD], BF16, name="wvb", tag="wvb")
            wo_sb = wp.tile([128, DT * MODEL], BF16, name="wo", tag="wo")
            bqk_sb = wp.tile([128, 2 * DT], F32, name="bqk", tag="bqk")
            cTi = [acp.tile([128, lc], BF16, name=f"cTi{a}", tag=f"cTi{a}")
                   for a in range(NT)]
            qTi = [acp.tile([128, lq], BF16, name=f"qTi{a}", tag=f"qTi{a}")
                   for a in range(NT)]
            ones_c = acp.tile([1, lc], BF16, name="ones_c", tag="ones_c")
            nc.vector.memset(ones_c[:], 1.0)

            kT = [acp.tile([128, lc], BF16, name=f"kT{t}", tag=f"kT{t}")
                  for t in range(DT)]
            qTs = [[acp.tile([128, QE], BF16, name=f"qTs{t}_{e}",
                             tag=f"qTs{t}_{e}") for e in range(NQE)]
                   for t in range(DT)]
            v_sb = [acp.tile([128, HPC * (D + 1)], BF16, name=f"v{ct}",
                             tag=f"v{ct}") for ct in range(CT)]
            attn = [[acp.tile([128, QE], BF16, name=f"attn{t}_{e}",
                              tag=f"attn{t}_{e}") for e in range(NQE)]
                    for t in range(DT)]

            # ---- input DMAs, priority order (arrival order = issue order).
            # wkq in chunks so kproj slab 0 starts after the first 128KB;
            # cT slab 0 ahead of qT/wv: kproj slab 0 gates the stream start.
            for a in range(NT):
                nc.sync.dma_start(wkq_sb[:, a * 512:(a + 1) * 512],
                                  wkq_d[:, a * 512:(a + 1) * 512])
            nc.sync.dma_start(bqk_sb[:], bqk_d[:, :])
            for a in range(NT):
                nc.sync.dma_start(cTi[a][:, 0:512],
                                  cT_d[a * 128:(a + 1) * 128, 0:512])
            for a in range(NT):
                nc.sync.dma_start(qTi[a][:, 0:QE], qT_d[a * 128:(a + 1) * 128,
                                                        0:QE])
            nc.sync.dma_start(wv_sb[:], wv_d[:, :])
            nc.sync.dma_start(wvb_sb[:], wvb_d[:, :])
            # remaining cT in 512-col slabs so kproj/vproj chase arrivals.
            for s in range(1, CS):
                for a in range(NT):
                    nc.sync.dma_start(cTi[a][:, s * 512:(s + 1) * 512],
                                      cT_d[a * 128:(a + 1) * 128,
                                           s * 512:(s + 1) * 512])
            nc.sync.dma_start(wo_sb[:], wo_d[:, :])
            for a in range(NT):
                nc.sync.dma_start(qTi[a][:, QE:lq],
                                  qT_d[a * 128:(a + 1) * 128, QE:lq])

            def wk_ap(a, t):
                return wkq_sb[:, a * 512 + t * 128:a * 512 + (t + 1) * 128]

            def wq_ap(a, t):
                return wkq_sb[:, a * 512 + 256 + t * 128:
                              a * 512 + 256 + (t + 1) * 128]

            def emit_all(reps_i):
                # producer gens (qproj/kproj) — must complete before their
                # consumer block's scores are EMITTED (in-order PE queue).
                prod = deque()
                # consumer gens (outproj) — their LDWEIGHTS reads attn, so
                # drain them only well after the normalize that writes it.
                late = deque()

                # Credit-based, GEN-ATOMIC drains: once a generator starts,
                # run it to completion (credit may go negative and is repaid
                # by later pumps).  A generator left mid-flight holds an sml
                # PSUM slot; any other sml allocation would then reuse that
                # slot and wait on the held gen's later-emitted reads — a
                # head-of-line deadlock in the in-order PE queue (v3.2 bug:
                # NRT_EXEC_UNIT_UNRECOVERABLE).
                credit = {"prod": 0, "late": 0}

                def drain(q, key, n):
                    if not q:
                        credit[key] = 0
                        return
                    credit[key] += n
                    while credit[key] > 0 and q:
                        g = q[0]
                        try:
                            while True:
                                next(g)
                                credit[key] -= 1
                        except StopIteration:
                            q.popleft()

                def qproj(t, e):
                    """q-projection for (d-tile t, q-block e), inline."""
                    ps = sml.tile([128, QE], F32, name="sml_ps", tag="sml")
                    for a in range(NT):
                        nc.tensor.matmul(
                            ps[:], wq_ap(a, t),
                            qTi[a][:, e * QE:(e + 1) * QE],
                            start=(a == 0), stop=(a == NT - 1))
                    nc.vector.tensor_scalar_add(
                        qTs[t][e][:], ps[:], bqk_sb[:, t:t + 1])

                def gen_qproj(t, e):
                    ps = sml.tile([128, QE], F32, name="sml_ps", tag="sml")
                    for a in range(NT):
                        nc.tensor.matmul(
                            ps[:], wq_ap(a, t),
                            qTi[a][:, e * QE:(e + 1) * QE],
                            start=(a == 0), stop=(a == NT - 1))
                        yield
                    nc.vector.tensor_scalar_add(
                        qTs[t][e][:], ps[:], bqk_sb[:, t:t + 1])

                def kproj_slab(t, qb):
                    """k-projection d-tile t for 512-col slab qb, inline."""
                    ps = sml.tile([128, 512], F32, name="sml_ps", tag="sml")
                    for a in range(NT):
                        nc.tensor.matmul(
                            ps[:], wk_ap(a, t),
                            cTi[a][:, qb * 512:(qb + 1) * 512],
                            start=(a == 0), stop=(a == NT - 1))
                    nc.vector.tensor_scalar_add(
                        kT[t][:, qb * 512:(qb + 1) * 512], ps[:],
                        bqk_sb[:, DT + t:DT + t + 1])

                def gen_kproj_slab(t, qb):
                    ps = sml.tile([128, 512], F32, name="sml_ps", tag="sml")
                    for a in range(NT):
                        nc.tensor.matmul(
                            ps[:], wk_ap(a, t),
                            cTi[a][:, qb * 512:(qb + 1) * 512],
                            start=(a == 0), stop=(a == NT - 1))
                        yield
                    nc.vector.tensor_scalar_add(
                        kT[t][:, qb * 512:(qb + 1) * 512], ps[:],
                        bqk_sb[:, DT + t:DT + t + 1])

                def vproj_half(ct, hp):
                    """v-projection for context chunk ct, head pair hp."""
                    HHD = 128  # two heads * 64 d
                    ps = sml.tile([128, 512], F32, name="sml_ps", tag="sml")
                    for a in range(NT):
                        nc.tensor.matmul(
                            ps[:, 0:HHD], cTi[a][:, ct * 128:(ct + 1) * 128],
                            wv_sb[:, a * HD + hp * HHD:
                                  a * HD + (hp + 1) * HHD],
                            start=(a == 0), stop=False)
                    nc.tensor.matmul(
                        ps[:, 0:HHD], ones_c[:, ct * 128:(ct + 1) * 128],
                        wvb_sb[:, hp * HHD:(hp + 1) * HHD],
                        start=False, stop=True)
                    vg = v_sb[ct].rearrange("p (g x) -> p g x", x=D + 1)
                    nc.vector.tensor_copy(
                        vg[:, 2 * hp:2 * hp + 2, 0:D],
                        ps[:, 0:HHD].rearrange("p (g x) -> p g x", x=D))
                    nc.vector.memset(vg[:, 2 * hp:2 * hp + 2, D:D + 1], 1.0)

                def gen_outproj(e, qt):
                    """out-projection for query tile qt of q-block e."""
                    ostg = osb.tile([128, MODEL], BF16, name="ostg", tag="ostg")
                    for nh in range(MODEL // 512):
                        ps = sml.tile([128, 512], F32, name="sml_ps",
                                      tag="sml")
                        for t in range(DT):
                            nc.tensor.matmul(
                                ps[:], attn[t][e][:, qt * 128:(qt + 1) * 128],
                                wo_sb[:, t * MODEL + nh * 512:
                                      t * MODEL + (nh + 1) * 512],
                                start=(t == 0), stop=(t == DT - 1))
                            yield
                        nc.vector.tensor_copy(
                            ostg[:, nh * 512:(nh + 1) * 512], ps[:])
                    nc.sync.dma_start(
                        out_d[(e * QE + qt * 128):(e * QE + (qt + 1) * 128), :],
                        ostg[:])

                def gen_outproj1(e, qt, t, dst, row0):
                    """single-t out-projection partial for query tile qt."""
                    ostg = osb.tile([128, MODEL], BF16, name="ostg", tag="ostg")
                    for nh in range(MODEL // 512):
                        ps = sml.tile([128, 512], F32, name="sml_ps",
                                      tag="sml")
                        nc.tensor.matmul(
                            ps[:], attn[t][e][:, qt * 128:(qt + 1) * 128],
                            wo_sb[:, t * MODEL + nh * 512:
                                  t * MODEL + (nh + 1) * 512],
                            start=True, stop=True)
                        yield
                        nc.vector.tensor_copy(
                            ostg[:, nh * 512:(nh + 1) * 512], ps[:])
                    nc.sync.dma_start(
                        dst[(row0 + qt * 128):(row0 + (qt + 1) * 128), :],
                        ostg[:])

                # ---------- ramp: k-proj t0 slab 0 first (gates stream),
                # then q-proj (t0, e0), then remaining k-proj t0 slabs.
                kproj_slab(0, 0)
                qproj(0, 0)
                for qb in range(1, CS):
                    kproj_slab(0, qb)

                # ---------- producer queue (drained in PE slack) ----------
                # hp0 phase consumers: qTs[0][e] (e>=1), kT[1], qTs[1][*]
                for e in range(1, NQE):
                    prod.append(gen_qproj(0, e))
                for qb in range(CS):
                    prod.append(gen_kproj_slab(1, qb))
                prod.append(gen_qproj(1, 0))
                for e in range(1, NQE):
                    prod.append(gen_qproj(1, e))

                def norm_step(hp, e, asb, h, c):
                    """One normalize micro-step: 128-col chunk c of head h.
                    Chunked so (a) the scheduler's ~12x-optimistic reciprocal
                    cost model can only misplace ~1us of dependent work, and
                    (b) each outproj qt tile depends ONLY on its own chunk."""
                    f0, f1 = c * 128, (c + 1) * 128
                    def step():
                        rz = nrm.tile([1, 128], BF16, name="rz", tag="rz")
                        with nc.allow_low_precision(
                                reason="1/Z in bf16 (~0.4%), inside the "
                                       "2e-2 budget"):
                            nc.vector.reciprocal(rz[:], asb[64:65, f0:f1])
                        rb = nrm.tile([64, 128], BF16, name="rb", tag="rb")
                        nc.gpsimd.partition_broadcast(rb[:], rz[:])
                        if h == 0:
                            nc.vector.tensor_mul(
                                attn[hp][e][0:64, f0:f1], asb[0:64, f0:f1],
                                rb[:])
                        else:
                            at = atp.tile([64, 128], BF16, name="at",
                                          tag="at")
                            nc.vector.tensor_mul(at[:], asb[0:64, f0:f1],
                                                 rb[:])
                            nc.sync.dma_start(attn[hp][e][64:128, f0:f1],
                                              at[:])
                    return step

                norm_q = deque()   # pending normalize micro-steps (FIFO)

                # ---------- attention stream: hp outer, q-blocks inner ----
                for hp in range(DT):
                    for e in range(NQE):
                        av = [avp.tile([65, QE], F32, name=f"av{h}",
                                       tag=f"av{h}") for h in range(2)]
                        pend = deque()   # (pt, ct) awaiting AV emission

                        def emit_av(hp=hp, av=av, pend=pend):
                            ppt, pct = pend.popleft()
                            for h in range(2):
                                nc.tensor.matmul(
                                    av[h][:],
                                    v_sb[pct][:, (2 * hp + h) * (D + 1):
                                              (2 * hp + h + 1) * (D + 1)],
                                    ppt[:, h * QE:(h + 1) * QE],
                                    start=(pct == 0), stop=(pct == CT - 1))

                        for ct in range(CT):
                            # deferred normalize micro-steps, 2/ct once the
                            # boundary-critical DVE work is behind us (not
                            # in e==0 blocks: vproj owns the DVE there)
                            if e > 0 and ct >= 2:
                                for _ in range(2):
                                    if norm_q:
                                        norm_q.popleft()()
                            st = stp.tile([128, 2 * QE], F32, name="st",
                                          tag="st")
                            nc.tensor.matmul(
                                st[:, 0:QE],
                                kT[hp][0:64, ct * 128:(ct + 1) * 128],
                                qTs[hp][e][0:64, :],
                                start=True, stop=True)
                            nc.tensor.matmul(
                                st[:, QE:2 * QE],
                                kT[hp][64:128, ct * 128:(ct + 1) * 128],
                                qTs[hp][e][64:128, :],
                                start=True, stop=True)
                            pt = ptp.tile([128, 2 * QE], BF16, name="pt",
                                          tag="pt")
                            nc.scalar.activation(pt[:], st[:], EXPF,
                                                 scale=SCALE)
                            # JIT v-projection inside the first q-block of
                            # each hp phase (AV(ct) trails by 4 steps).
                            if e == 0:
                                vproj_half(ct, hp)
                            pend.append((pt, ct))
                            if len(pend) > 4:
                                emit_av()
                            if e > 0 and ct < 8:
                                drain(prod, "prod", 2)
                        # Boundary fillers BEFORE the trailing-AV drain: the
                        # last AVs wait on the last exps (ACT-paced), so
                        # ready outproj/producer matmuls fill that bubble.
                        drain(late, "late", 40)
                        drain(prod, "prod", 10)
                        while pend:
                            emit_av()
                        # ---- normalize pair: copies NOW (free the PSUM av
                        # banks in ~1.2us); recip/bcast/mul deferred as
                        # chunked micro-steps, interleaved h0/h1 per chunk
                        # so outproj qt tiles unblock in qt order.
                        asbs = []
                        for h in range(2):
                            asb = avs.tile([65, QE], BF16, name="asb",
                                           tag="asb")
                            nc.vector.tensor_copy(asb[:], av[h][:])
                            asbs.append(asb)
                        for c in range(QE // 128):
                            for h in range(2):
                                norm_q.append(norm_step(hp, e, asbs[h], h, c))
                        # after pair 1 of block e: queue its out-projection.
                        # The last q-block is split: its t0 half (attn[0]
                        # normalized back in the hp0 phase) is queued early
                        # and runs mid-stream; only the t1 half remains for
                        # the tail (host adds the outp2 partial).
                        if hp == DT - 1:
                            if e < NQE - 1:
                                for qt in range(QE // 128):
                                    late.append(gen_outproj(e, qt))
                                if e == 1:
                                    for qt in range(QE // 128):
                                        late.append(gen_outproj1(
                                            NQE - 1, qt, 0, out_d,
                                            (NQE - 1) * QE))
                            else:
                                for qt in range(QE // 128):
                                    late.append(gen_outproj1(
                                        NQE - 1, qt, 1, out2_d, 0))

                # ---------- tail ----------
                while norm_q:
                    norm_q.popleft()()
                drain(late, "late", 10 ** 9)
                drain(prod, "prod", 10 ** 9)

            for r in range(reps):
                emit_all(r)

    nc.compile()
    return nc


def _prep_inputs(query, context, Wq, bq, Wkv, bkv, Wout, lq, lc):
    """Host-side shard/cast/pack. Returns in_maps for 8 cores."""
    NT = MODEL // 128
    DT = HD // 128
    Wkv_r = np.asarray(Wkv, np.float32).reshape(MODEL, H, D, 2)
    bkv_r = np.asarray(bkv, np.float32).reshape(H, D, 2)
    Wq = np.asarray(Wq, np.float32)
    bq = np.asarray(bq, np.float32)
    Wout = np.asarray(Wout, np.float32)

    qT = [np.ascontiguousarray(np.asarray(query[b], np.float32).T).astype(NPBF16)
          for b in range(B)]
    cT = [np.ascontiguousarray(np.asarray(context[b], np.float32).T).astype(NPBF16)
          for b in range(B)]

    grp = []
    for g in range(GROUPS):
        hs = slice(g * HPC, (g + 1) * HPC)
        wq_c = Wq[:, g * HD:(g + 1) * HD]                      # [M, HD]
        wk_c = Wkv_r[:, hs, :, 0].reshape(MODEL, HD)           # [M, HD]
        wv_c = Wkv_r[:, hs, :, 1].reshape(MODEL, HD)           # [M, HD]
        bv_c = bkv_r[hs, :, 1].reshape(1, HD)
        wo_c = Wout[g * HD:(g + 1) * HD, :]                    # [HD, M]
        bq_c = bq[g * HD:(g + 1) * HD]
        bk_c = bkv_r[hs, :, 0].reshape(HD)

        # wkq packed [128, NT*512]: chunk a -> [wk_a | wq_a]
        wkq = np.empty((128, NT * 512), np.float32)
        for a in range(NT):
            wkq[:, a * 512:a * 512 + 256] = wk_c[a * 128:(a + 1) * 128, :]
            wkq[:, a * 512 + 256:(a + 1) * 512] = wq_c[a * 128:(a + 1) * 128, :]
        # wv packed [128, NT*HD]: chunk a at cols a*HD
        wvp = np.empty((128, NT * HD), np.float32)
        for a in range(NT):
            wvp[:, a * HD:(a + 1) * HD] = wv_c[a * 128:(a + 1) * 128, :]
        # wo packed [128, DT*MODEL]: d-tile t at cols t*MODEL
        wop = np.empty((128, DT * MODEL), np.float32)
        for t in range(DT):
            wop[:, t * MODEL:(t + 1) * MODEL] = wo_c[t * 128:(t + 1) * 128, :]
        # biases [128, 2*DT]: cols [bq_t0, bq_t1, bk_t0, bk_t1]
        bqk = np.empty((128, 2 * DT), np.float32)
        for t in range(DT):
            bqk[:, t] = bq_c[t * 128:(t + 1) * 128]
            bqk[:, DT + t] = bk_c[t * 128:(t + 1) * 128]
        grp.append((wkq.astype(NPBF16), wvp.astype(NPBF16),
                    np.ascontiguousarray(bv_c, np.float32).astype(NPBF16),
                    wop.astype(NPBF16), bqk))

    in_maps = []
    for c in range(N_CORES):
        b, g = c // GROUPS, c % GROUPS
        wkq, wvp, wvb, wop, bqk = grp[g]
        in_maps.append({
            "qT": qT[b], "cT": cT[b],
            "wkq": wkq, "wv": wvp, "wvb": wvb, "wo": wop, "bqk": bqk,
        })
    return in_maps


def _reduce_out(results, bout, lq):
    bout = np.asarray(bout, np.float32)
    out = np.empty((B, lq, MODEL), np.float32)
    for b in range(B):
        acc = None
        for g in range(GROUPS):
            r = results[b * GROUPS + g]
            part = r["outp"].astype(np.float32)
            # last q-block's t1-partial is shipped separately (outp2)
            part[-r["outp2"].shape[0]:] += r["outp2"].astype(np.float32)
            acc = part if acc is None else acc + part
        out[b] = acc + bout
    return out


class Runner:
    """Cached-jit PJRT executor for the SPMD bass kernel (axon path).

    Mirrors bass2jax.run_bass_via_pjrt's multi-core branch, but builds the
    jitted sharded callable once so repeated calls reuse the compiled
    executable (for steady-state timing) and inputs can be pre-placed on
    device.
    """

    def __init__(self, nc, n_cores=N_CORES):
        import jax
        from jax.sharding import Mesh, PartitionSpec, NamedSharding
        from jax.experimental.shard_map import shard_map

        bass2jax.install_neuronx_cc_hook()
        assert nc.dbg_addr is None
        part_name = (nc.partition_id_tensor.name
                     if nc.partition_id_tensor else None)

        in_names, out_names, out_avals, zero_outs = [], [], [], []
        for alloc in nc.m.functions[0].allocations:
            if not isinstance(alloc, mybir.MemoryLocationSet):
                continue
            name = alloc.memorylocations[0].name
            if alloc.kind == "ExternalInput":
                if name != part_name:
                    in_names.append(name)
            elif alloc.kind == "ExternalOutput":
                shape = tuple(alloc.tensor_shape)
                dtype = mybir.dt.np(alloc.dtype)
                out_names.append(name)
                out_avals.append(jax.core.ShapedArray(shape, dtype))
                zero_outs.append(np.zeros(shape, dtype))
        self.n_params = len(in_names)
        self.in_names = list(in_names)
        self.out_names = out_names
        self.out_avals = out_avals
        self.zero_outs = zero_outs
        all_names = tuple(
            in_names + out_names + ([part_name] if part_name else []))

        def _body(*args):
            operands = list(args)
            if part_name is not None:
                operands.append(bass2jax.partition_id_tensor())
            outs = bass2jax._bass_exec_p.bind(
                *operands,
                out_avals=tuple(out_avals),
                in_names=all_names,
                out_names=tuple(out_names),
                lowering_input_output_aliases=(),
                sim_require_finite=True,
                sim_require_nnan=True,
                nc=nc,
            )
            return tuple(outs)

        devices = jax.devices()[:n_cores]
        assert len(devices) == n_cores
        self.n_cores = n_cores
        self.mesh = Mesh(np.asarray(devices), ("core",))
        self.sharding = NamedSharding(self.mesh, PartitionSpec("core"))
        n_args = self.n_params + len(out_names)
        self.fn = jax.jit(
            shard_map(_body, mesh=self.mesh,
                      in_specs=(PartitionSpec("core"),) * n_args,
                      out_specs=(PartitionSpec("core"),) * len(out_names),
                      check_rep=False),
            keep_unused=True,
        )
        self._jax = jax

    def put(self, in_maps):
        """Concatenate per-core inputs on axis 0 and place on devices."""
        jax = self._jax
        args = []
        for name in self.in_names:
            arr = np.concatenate(
                [np.asarray(m[name]) for m in in_maps], axis=0)
            args.append(jax.device_put(arr, self.sharding))
        for z in self.zero_outs:
            zz = np.zeros((self.n_cores * z.shape[0], *z.shape[1:]), z.dtype)
            args.append(jax.device_put(zz, self.sharding))
        return args

    def call(self, dev_args):
        outs = self.fn(*dev_args)
        self._jax.block_until_ready(outs)
        return outs

    def gather(self, outs):
        """outs -> list (per core) of {name: np.ndarray}."""
        res = []
        for c in range(self.n_cores):
            d = {}
            for i, name in enumerate(self.out_names):
                full = np.asarray(outs[i])
                d[name] = full.reshape(
                    self.n_cores, *self.out_avals[i].shape)[c]
            res.append(d)
        return res


_CACHE = {}


def _get_runner(lq, lc):
    key = (lq, lc)
    if key not in _CACHE:
        _CACHE[key] = Runner(build_nc(lq, lc))
    return _CACHE[key]


def run(query, context, Wq, bq, Wkv, bkv, Wout, bout):
    lq, lc = query.shape[1], context.shape[1]
    runner = _get_runner(lq, lc)
    in_maps = _prep_inputs(query, context, Wq, bq, Wkv, bkv, Wout, lq, lc)
    dev_args = runner.put(in_maps)
    outs = runner.call(dev_args)
    results = runner.gather(outs)
    return _reduce_out(results, bout, lq), runner, dev_args


def kernel(query, context, Wq, bq, Wkv, bkv, Wout, bout):
    out, _, _ = run(query, context, Wq, bq, Wkv, bkv, Wout, bout)
    return out
